# revision 6
# baseline (speedup 1.0000x reference)
"""Causal self-attention with RoPE, tensor-parallel over heads on 8 TRN2 NeuronCores.

Model (from the reference):
    q/k/v = x @ W{q,k,v}.T          x: (1, 2048, 2048), 16 heads x 128 head_dim
    rope(q), rope(k)                half-rotation, 32 nonzero freqs
    causal softmax(q k^T / sqrt(128)) @ v
    out = (y / 3) @ Wo.T

Sharding: 2 heads per core. Each core computes its heads' q/k/v projections,
attention, and a partial c_proj (its 256 columns of the hd contraction);
the host sums the 8 partial outputs (the "all-reduce after c_proj").

Per-core kernel layout choices:
  - Everything transposed so the contraction dim is always on partitions:
    host supplies xT (D, T) plus pre-transposed weight slices.
  - Scores computed transposed (S^T[j, i] blocks) so the P @ V matmul needs
    no transposes: OT[d, i] = sum_j V[j, d]^T P^T[j, i] is produced directly
    in the layout c_proj wants.
  - Softmax without max-subtraction (scores are provably tiny: |s| < ~2),
    denominator via DVE accumulation + one all-ones matmul (broadcast sum).
  - RoPE in transposed layout via a 64-partition roll matmul + 3 DVE ops.
  - All matmuls in float32r (full PE rate at moving dim >= 256).
"""

import numpy as np

T = 2048
D = 2048
H = 16
DH = 128
N_CORES = 8
H_LOC = H // N_CORES          # heads per core = 2
HD_LOC = H_LOC * DH           # local head dims = 256
TCH = 512                     # query-chunk width
N_CH = T // TCH               # 4 chunks
KO = D // 128                 # 16 contraction subtiles
XP = 4                        # xT streamed in pieces of 4 k-subtiles
SCALE = (DH ** 0.5) / DH      # 1/sqrt(128)

_CACHE = {}


def build_program():
    """Build (once) the single-core Bass program shared by all 8 cores."""
    if "nc" in _CACHE:
        return _CACHE["nc"]

    from contextlib import ExitStack

    import concourse.bacc as bacc
    import concourse.mybir as mybir
    import concourse.tile as tile

    f32 = mybir.dt.float32
    f32r = mybir.dt.float32r
    EXP = mybir.ActivationFunctionType.Exp

    nc = bacc.Bacc("TRN2", target_bir_lowering=False)

    xT_d = nc.dram_tensor("xT", (D, T), f32r, kind="ExternalInput")
    wq_d = nc.dram_tensor("wqT", (D, HD_LOC), f32r, kind="ExternalInput")
    wk_d = nc.dram_tensor("wkT", (D, HD_LOC), f32r, kind="ExternalInput")
    wv_d = nc.dram_tensor("wvT", (D, HD_LOC), f32r, kind="ExternalInput")
    wo_d = nc.dram_tensor("woT", (HD_LOC, D), f32r, kind="ExternalInput")
    ct_d = nc.dram_tensor("ctab", (128, T), f32, kind="ExternalInput")
    st_d = nc.dram_tensor("stab", (128, T), f32, kind="ExternalInput")
    roll_d = nc.dram_tensor("roll", (128, 128), f32r, kind="ExternalInput")
    ones_d = nc.dram_tensor("ones", (128, 128), f32r, kind="ExternalInput")
    tri_d = nc.dram_tensor("tri", (128, 128), f32r, kind="ExternalInput")
    out_d = nc.dram_tensor("outp", (T, D), f32, kind="ExternalOutput")

    xT_r = xT_d[:].rearrange("(ko p) t -> p ko t", p=128)
    wq_r = wq_d[:].rearrange("(ko p) m -> p ko m", p=128)
    wk_r = wk_d[:].rearrange("(ko p) m -> p ko m", p=128)
    wv_r = wv_d[:].rearrange("(ko p) m -> p ko m", p=128)
    wo_r = wo_d[:].rearrange("(h p) d -> p h d", p=128)

    with tile.TileContext(nc) as tc, ExitStack() as ctx:
        persist = ctx.enter_context(tc.tile_pool(name="persist", bufs=1))
        xpool = ctx.enter_context(tc.tile_pool(name="xpool", bufs=3))
        ptpool = ctx.enter_context(tc.tile_pool(name="ptpool", bufs=4))
        rtmp = ctx.enter_context(tc.tile_pool(name="rtmp", bufs=2))
        spool = ctx.enter_context(tc.tile_pool(name="spool", bufs=2))
        opool = ctx.enter_context(tc.tile_pool(name="opool", bufs=4))
        psum = ctx.enter_context(tc.tile_pool(name="psum", bufs=8, space="PSUM"))

        def ps_tile():
            return psum.tile([128, TCH], f32, tag="ps", name="ps")

        # --- resident tensors ---
        w_q = persist.tile([128, KO, HD_LOC], f32r, tag="w_q")
        w_k = persist.tile([128, KO, HD_LOC], f32r, tag="w_k")
        w_v = persist.tile([128, KO, HD_LOC], f32r, tag="w_v")
        w_o = persist.tile([128, H_LOC, D], f32r, tag="w_o")
        qt = persist.tile([128, H_LOC, T], f32r, tag="qt")
        kt = persist.tile([128, H_LOC, T], f32r, tag="kt")
        vt = persist.tile([128, KO, HD_LOC], f32r, tag="vt")
        yt = persist.tile([128, H_LOC, T], f32r, tag="yt")
        ctab = persist.tile([128, T], f32, tag="ctab")
        stab = persist.tile([128, T], f32, tag="stab")
        roll = persist.tile([128, 128], f32r, tag="roll")
        ones = persist.tile([128, 128], f32r, tag="ones")
        tri = persist.tile([128, 128], f32r, tag="tri")

        nc.sync.dma_start(w_q[:], wq_r)
        nc.sync.dma_start(w_k[:], wk_r)
        nc.sync.dma_start(w_v[:], wv_r)
        nc.sync.dma_start(w_o[:], wo_r)
        nc.sync.dma_start(ctab[:], ct_d[:])
        nc.sync.dma_start(stab[:], st_d[:])
        nc.sync.dma_start(roll[:], roll_d[:])
        nc.sync.dma_start(ones[:], ones_d[:])
        nc.sync.dma_start(tri[:], tri_d[:])

        for c in range(N_CH):
            cs = c * TCH
            # --- stream this t-chunk of xT (all of K) in XP-wide pieces ---
            pieces = []
            for kp in range(KO // XP):
                xc = xpool.tile([128, XP, TCH], f32r, tag="xc")
                nc.sync.dma_start(
                    xc[:], xT_r[:, kp * XP:(kp + 1) * XP, cs:cs + TCH]
                )
                pieces.append(xc)

            # --- q/k projections: psum[m, t] = sum_k W[k, m] x[k, t] ---
            for w_sb, dst in ((w_q, qt), (w_k, kt)):
                for h in range(H_LOC):
                    ps = ps_tile()
                    for ko in range(KO):
                        nc.tensor.matmul(
                            ps,
                            lhsT=w_sb[:, ko, h * 128:(h + 1) * 128],
                            rhs=pieces[ko // XP][:, ko % XP, :],
                            start=(ko == 0),
                            stop=(ko == KO - 1),
                        )
                    nc.vector.tensor_copy(out=dst[:, h, cs:cs + TCH], in_=ps)

            # --- v projection (natural layout): psum[t, n] = sum_k x[k, t] W[k, n] ---
            for tt in range(TCH // 128):
                gt = c * (TCH // 128) + tt
                ps = ps_tile()
                for ko in range(KO):
                    nc.tensor.matmul(
                        ps[:, :HD_LOC],
                        lhsT=pieces[ko // XP][:, ko % XP, tt * 128:(tt + 1) * 128],
                        rhs=w_v[:, ko, :],
                        start=(ko == 0),
                        stop=(ko == KO - 1),
                    )
                nc.scalar.copy(out=vt[:, gt, :], in_=ps[:, :HD_LOC])

            # --- RoPE on this chunk of qT / kT ---
            for src in (qt, kt):
                for h in range(H_LOC):
                    sl = src[:, h, cs:cs + TCH]
                    rolled = ps_tile()
                    nc.tensor.matmul(rolled, lhsT=roll, rhs=sl,
                                     start=True, stop=True)
                    a = rtmp.tile([128, TCH], f32, tag="ra")
                    b = rtmp.tile([128, TCH], f32, tag="rb")
                    nc.vector.tensor_mul(out=a, in0=sl, in1=ctab[:, cs:cs + TCH])
                    nc.vector.tensor_mul(out=b, in0=rolled, in1=stab[:, cs:cs + TCH])
                    nc.vector.tensor_add(out=sl, in0=a, in1=b)

            # --- attention for query chunk c, both heads ---
            n_jt = 4 * c + 4
            for h in range(H_LOC):
                ot = ps_tile()          # OT[d, i] accumulator
                vecsum = spool.tile([128, TCH], f32r, tag="vecsum")
                for jt in range(n_jt):
                    st_ps = ps_tile()
                    nc.tensor.matmul(
                        st_ps,
                        lhsT=kt[:, h, jt * 128:(jt + 1) * 128],
                        rhs=qt[:, h, cs:cs + TCH],
                        start=True,
                        stop=True,
                    )
                    pt = ptpool.tile([128, TCH], f32r, tag="pt")
                    m = jt - 4 * c
                    # diagonal block: columns < 128m are fully causally
                    # masked -- never written, never read (partial-width ops)
                    lo = 128 * m if m > 0 else 0
                    nc.scalar.activation(
                        out=pt[:, lo:], in_=st_ps[:, lo:], func=EXP, scale=SCALE,
                    )
                    if m >= 0:
                        nc.vector.tensor_mul(
                            out=pt[:, 128 * m:128 * (m + 1)],
                            in0=pt[:, 128 * m:128 * (m + 1)],
                            in1=tri[:],
                        )
                    if jt == 0:
                        nc.vector.tensor_copy(out=vecsum[:], in_=pt[:])
                    else:
                        nc.vector.tensor_add(out=vecsum[:, lo:], in0=vecsum[:, lo:],
                                             in1=pt[:, lo:])
                    nc.tensor.matmul(
                        ot[:, lo:],
                        lhsT=vt[:, jt, h * 128:(h + 1) * 128],
                        rhs=pt[:, lo:],
                        start=(jt == 0),
                        stop=(jt == n_jt - 1),
                        skip_group_check=(lo > 0),
                    )
                # denominator: all-ones matmul -> column sums broadcast to all rows
                den = ps_tile()
                nc.tensor.matmul(den, lhsT=ones, rhs=vecsum[:],
                                 start=True, stop=True)
                recipb = spool.tile([128, TCH], f32, tag="recipb")
                nc.vector.reciprocal(out=recipb[:], in_=den)
                nc.vector.tensor_mul(out=yt[:, h, cs:cs + TCH], in0=ot, in1=recipb[:])

            # --- partial c_proj for this chunk's rows ---
            for tt in range(TCH // 128):
                gt = c * (TCH // 128) + tt
                for nck in range(D // 512):
                    ps = ps_tile()
                    for h in range(H_LOC):
                        nc.tensor.matmul(
                            ps,
                            lhsT=yt[:, h, gt * 128:(gt + 1) * 128],
                            rhs=w_o[:, h, nck * 512:(nck + 1) * 512],
                            start=(h == 0),
                            stop=(h == H_LOC - 1),
                        )
                    ob = opool.tile([128, 512], f32, tag="ob")
                    nc.scalar.copy(out=ob[:], in_=ps)
                    nc.sync.dma_start(
                        out_d[gt * 128:(gt + 1) * 128, nck * 512:(nck + 1) * 512],
                        ob[:],
                    )

    nc.compile()
    _CACHE["nc"] = nc
    return nc


def host_inputs(x, Wq, Wk, Wv, Wo):
    """Per-core input dicts (host-side shard + transpose + table prep)."""
    x2 = np.ascontiguousarray(x.reshape(T, D).T).astype(np.float32)  # (D, T)

    half = DH // 2  # 64
    af = (1.0 / 1024.0) ** np.linspace(0.0, 1.0, DH // 4, dtype=np.float32)
    af = np.concatenate([af, np.zeros(DH // 4, np.float32)])         # (64,)
    theta = np.arange(T, dtype=np.float32)[:, None] * af[None, :]    # (T, 64)
    cos = np.cos(theta).T.astype(np.float32)                         # (64, T)
    sin = np.sin(theta).T.astype(np.float32)
    ctab = np.concatenate([cos, cos], axis=0)                        # (128, T)
    stab = np.concatenate([sin, -sin], axis=0)

    roll = np.zeros((128, 128), np.float32)
    for p in range(128):
        roll[p, (p + half) % 128] = 1.0
    ones = np.ones((128, 128), np.float32)
    tri = np.triu(np.ones((128, 128), np.float32))  # tri[j, i] = i >= j

    shared = {
        "xT": x2, "ctab": ctab, "stab": stab,
        "roll": roll, "ones": ones, "tri": tri,
    }
    in_maps = []
    for c in range(N_CORES):
        sl = slice(c * HD_LOC, (c + 1) * HD_LOC)
        in_maps.append({
            **shared,
            "wqT": np.ascontiguousarray(Wq[sl, :].T),
            "wkT": np.ascontiguousarray(Wk[sl, :].T),
            "wvT": np.ascontiguousarray(Wv[sl, :].T),
            "woT": np.ascontiguousarray((Wo[:, sl] / 3.0).T),
        })
    return in_maps


def kernel(x, Wq, Wk, Wv, Wo):
    from concourse import bass_utils

    nc = build_program()
    in_maps = host_inputs(np.asarray(x), np.asarray(Wq), np.asarray(Wk),
                          np.asarray(Wv), np.asarray(Wo))
    res = bass_utils.run_bass_kernel_spmd(nc, in_maps, core_ids=list(range(N_CORES)))
    out = res.results[0]["outp"].astype(np.float64)
    for c in range(1, N_CORES):
        out += res.results[c]["outp"]
    return out.astype(np.float32).reshape(1, T, D)


# revision 26
# speedup vs baseline: 143676.9131x; 143676.9131x over previous
"""Causal self-attention with RoPE, tensor-parallel over heads on 8 TRN2 NeuronCores.

Model (from the reference):
    q/k/v = x @ W{q,k,v}.T          x: (1, 2048, 2048), 16 heads x 128 head_dim
    rope(q), rope(k)                half-rotation, 32 nonzero freqs
    causal softmax(q k^T / sqrt(128)) @ v
    out = (y / 3) @ Wo.T

Sharding: 2 heads per core. Each core computes its heads' q/k/v projections,
attention, and a partial c_proj (its 256 columns of the hd contraction);
the host sums the 8 partial outputs (the "all-reduce after c_proj").

Per-core kernel layout choices:
  - Everything transposed so the contraction dim is always on partitions:
    host supplies xT (D, T) plus pre-transposed weight slices.
  - Scores computed transposed (S^T[j, i] blocks) so the P @ V matmul needs
    no transposes: OT[d, i] = sum_j V[j, d]^T P^T[j, i] is produced directly
    in the layout c_proj wants.
  - Softmax without max-subtraction (scores are provably tiny: |s| < ~2),
    denominator via DVE accumulation + one all-ones matmul (broadcast sum).
  - RoPE in transposed layout via a 64-partition roll matmul + 3 DVE ops.
  - All matmuls in float32r (full PE rate at moving dim >= 256).
"""

import numpy as np

T = 2048
D = 2048
H = 16
DH = 128
N_CORES = 8
H_LOC = H // N_CORES          # heads per core = 2
HD_LOC = H_LOC * DH           # local head dims = 256
TCH = 512                     # query-chunk width
N_CH = T // TCH               # 4 chunks
KO = D // 128                 # 16 contraction subtiles
XP = 4                        # xT streamed in pieces of 4 k-subtiles
SCALE = (DH ** 0.5) / DH      # 1/sqrt(128)

_CACHE = {}


def build_program():
    """Build (once) the single-core Bass program shared by all 8 cores."""
    if "nc" in _CACHE:
        return _CACHE["nc"]

    from contextlib import ExitStack

    import concourse.bacc as bacc
    import concourse.mybir as mybir
    import concourse.tile as tile

    f32 = mybir.dt.float32
    f32r = mybir.dt.float32r
    bf16 = mybir.dt.bfloat16
    EXP = mybir.ActivationFunctionType.Exp

    nc = bacc.Bacc("TRN2", target_bir_lowering=False)

    xT_d = nc.dram_tensor("xT", (D, T), f32r, kind="ExternalInput")
    wq_d = nc.dram_tensor("wqT", (D, HD_LOC), f32r, kind="ExternalInput")
    wk_d = nc.dram_tensor("wkT", (D, HD_LOC), f32r, kind="ExternalInput")
    wv_d = nc.dram_tensor("wvT", (D, HD_LOC), f32r, kind="ExternalInput")
    wo_d = nc.dram_tensor("woT", (HD_LOC, D), f32r, kind="ExternalInput")
    ct_d = nc.dram_tensor("ctab", (128, T), f32, kind="ExternalInput")
    st_d = nc.dram_tensor("stab", (128, T), f32, kind="ExternalInput")
    roll_d = nc.dram_tensor("roll", (128, 128), f32r, kind="ExternalInput")
    ones_d = nc.dram_tensor("ones", (128, 128), f32r, kind="ExternalInput")
    tri_d = nc.dram_tensor("tri", (128, 128), f32r, kind="ExternalInput")
    out_d = nc.dram_tensor("outp", (T, D), f32, kind="ExternalOutput")

    xT_r = xT_d[:].rearrange("(ko p) t -> p ko t", p=128)
    wq_r = wq_d[:].rearrange("(ko p) m -> p ko m", p=128)
    wk_r = wk_d[:].rearrange("(ko p) m -> p ko m", p=128)
    wv_r = wv_d[:].rearrange("(ko p) m -> p ko m", p=128)
    wo_r = wo_d[:].rearrange("(h p) d -> p h d", p=128)

    with tile.TileContext(nc) as tc, ExitStack() as ctx:
        persist = ctx.enter_context(tc.tile_pool(name="persist", bufs=1))
        qpool = ctx.enter_context(tc.tile_pool(name="qpool", bufs=2))
        ypool = ctx.enter_context(tc.tile_pool(name="ypool", bufs=2))
        xpool = ctx.enter_context(tc.tile_pool(name="xpool", bufs=5))
        ptpool = ctx.enter_context(tc.tile_pool(name="ptpool", bufs=4))
        rtmp = ctx.enter_context(tc.tile_pool(name="rtmp", bufs=1))
        spool = ctx.enter_context(tc.tile_pool(name="spool", bufs=2))
        opool = ctx.enter_context(tc.tile_pool(name="opool", bufs=3))
        psum_p = ctx.enter_context(tc.tile_pool(name="psum_p", bufs=2, space="PSUM"))
        psum_mix = ctx.enter_context(tc.tile_pool(name="psum_mix", bufs=2, space="PSUM"))
        psum_ot = ctx.enter_context(tc.tile_pool(name="psum_ot", bufs=2, space="PSUM"))

        def ps_tile(pool=None):
            return (pool or psum_p).tile([128, TCH], f32, tag="ps", name="ps")

        def mix_tile():
            return psum_mix.tile([128, H_LOC, TCH], f32, tag="mix", name="mix")

        # --- resident tensors ---
        w_q = persist.tile([128, KO, HD_LOC], f32r, tag="w_q")
        w_k = persist.tile([128, KO, HD_LOC], f32r, tag="w_k")
        w_v = persist.tile([128, KO, HD_LOC], f32r, tag="w_v")
        w_o = persist.tile([128, H_LOC, D], f32r, tag="w_o")
        kt = persist.tile([128, H_LOC, T], f32r, tag="kt")
        vt = persist.tile([128, KO, HD_LOC], f32r, tag="vt")
        ctab = persist.tile([128, T], f32, tag="ctab")
        stab = persist.tile([128, T], f32, tag="stab")
        roll = persist.tile([128, 128], f32r, tag="roll")
        ones = persist.tile([128, 128], f32r, tag="ones")
        tri = persist.tile([128, 128], f32r, tag="tri")

        def proj_chunk(c):
            """q/k/v projections + RoPE for t-chunk c (xT streamed in pieces)."""
            cs = c * TCH
            pieces = []
            for kp in range(KO // XP):
                ksl = slice(kp * XP, (kp + 1) * XP)
                xc = xpool.tile([128, XP, TCH], f32r, tag="xc", name="xc")
                nc.sync.dma_start(xc[:], xT_r[:, ksl, cs:cs + TCH])
                pieces.append(xc)
                if c == 0:
                    nc.sync.dma_start(w_q[:, ksl, :], wq_r[:, ksl, :])
                    nc.sync.dma_start(w_k[:, ksl, :], wk_r[:, ksl, :])
                    nc.sync.dma_start(w_v[:, ksl, :], wv_r[:, ksl, :])
            if c == 0:
                nc.sync.dma_start(ctab[:], ct_d[:])
                nc.sync.dma_start(stab[:], st_d[:])
                nc.sync.dma_start(roll[:], roll_d[:])
                nc.sync.dma_start(ones[:], ones_d[:])
                nc.sync.dma_start(tri[:], tri_d[:])

            qc = qpool.tile([128, H_LOC, TCH], f32r, tag="qc", name="qc")
            for w_sb, dst in ((w_q, qc), (w_k, kt)):
                for h in range(H_LOC):
                    dsl = dst[:, h, :] if dst is qc else dst[:, h, cs:cs + TCH]
                    ps = ps_tile()
                    for ko in range(KO):
                        nc.tensor.matmul(
                            ps,
                            lhsT=w_sb[:, ko, h * 128:(h + 1) * 128],
                            rhs=pieces[ko // XP][:, ko % XP, :],
                            start=(ko == 0),
                            stop=(ko == KO - 1),
                        )
                    nc.scalar.copy(out=dsl, in_=ps)

            for tt in range(TCH // 128):
                gt = c * (TCH // 128) + tt
                ps = ps_tile()
                for ko in range(KO):
                    nc.tensor.matmul(
                        ps[:, :HD_LOC],
                        lhsT=pieces[ko // XP][:, ko % XP, tt * 128:(tt + 1) * 128],
                        rhs=w_v[:, ko, :],
                        start=(ko == 0),
                        stop=(ko == KO - 1),
                    )
                nc.scalar.copy(out=vt[:, gt, :], in_=ps[:, :HD_LOC])

            # RoPE: y = x*C + roll64(x)*S' (only via PE roll + 3 DVE ops)
            for srct in (qc, kt):
                for h in range(H_LOC):
                    sl = srct[:, h, :] if srct is qc else srct[:, h, cs:cs + TCH]
                    rolled = ps_tile()
                    nc.tensor.matmul(rolled, lhsT=roll, rhs=sl,
                                     start=True, stop=True)
                    a = rtmp.tile([128, TCH], f32, tag="ra", name="ra")
                    b = rtmp.tile([128, TCH], f32, tag="rb", name="rb")
                    nc.vector.tensor_mul(out=a, in0=sl, in1=ctab[:, cs:cs + TCH])
                    nc.vector.tensor_mul(out=b, in0=rolled, in1=stab[:, cs:cs + TCH])
                    nc.vector.tensor_add(out=sl, in0=a, in1=b)
            return qc

        def attn_chunk(c, qc):
            """Causal attention for query chunk c, heads interleaved."""
            cs = c * TCH
            yc = ypool.tile([128, H_LOC, TCH], f32r, tag="yc", name="yc")
            n_jt = 4 * c + 4
            ots = [ps_tile(psum_ot) for _ in range(H_LOC)]
            vecsums = [[spool.tile([128, TCH], f32r, tag=f"vecsum{par}",
                                   name="vecsum")
                        for par in range(2)] for _ in range(H_LOC)]
            for jt in range(n_jt):
                pair = mix_tile()
                for h in range(H_LOC):
                    nc.tensor.matmul(
                        pair[:, h, :],
                        lhsT=kt[:, h, jt * 128:(jt + 1) * 128],
                        rhs=qc[:, h, :],
                        start=True,
                        stop=True,
                    )
                pt = ptpool.tile([128, H_LOC, TCH], f32r, tag="pt", name="pt")
                m = jt - 4 * c
                # diagonal block: cols < 128m fully masked -- never written,
                # never read (partial-width ops)
                lo = 128 * m if m > 0 else 0
                if lo == 0:
                    # both heads in ONE activation call (contiguous 1024 wide)
                    nc.scalar.activation(out=pt[:, :, :], in_=pair[:, :, :],
                                         func=EXP, scale=SCALE)
                else:
                    for h in range(H_LOC):
                        nc.scalar.activation(out=pt[:, h, lo:],
                                             in_=pair[:, h, lo:],
                                             func=EXP, scale=SCALE)
                for h in range(H_LOC):
                    if m >= 0:
                        nc.vector.tensor_mul(
                            out=pt[:, h, 128 * m:128 * (m + 1)],
                            in0=pt[:, h, 128 * m:128 * (m + 1)],
                            in1=tri[:],
                        )
                    # chunk 0: jt==1 is diagonal (cols < 128 unwritten), so a
                    # full-width init copy would ingest garbage -- use a single
                    # DVE accumulator there. Other chunks split DVE/GPSIMD.
                    par = jt % 2 if c > 0 else 0
                    vs = vecsums[h][par]
                    eng = nc.vector if par == 0 else nc.gpsimd
                    if jt < (2 if c > 0 else 1):
                        eng.tensor_copy(out=vs[:], in_=pt[:, h, :])
                    else:
                        eng.tensor_add(out=vs[:, lo:], in0=vs[:, lo:],
                                       in1=pt[:, h, lo:])
                    nc.tensor.matmul(
                        ots[h][:, lo:],
                        lhsT=vt[:, jt, h * 128:(h + 1) * 128],
                        rhs=pt[:, h, lo:],
                        start=(jt == 0),
                        stop=(jt == n_jt - 1),
                        skip_group_check=(lo > 0),
                    )
            for h in range(H_LOC):
                # denominator: all-ones matmul -> column sums on all partitions
                den = mix_tile()[:, 0, :]
                if c > 0:
                    nc.tensor.matmul(den, lhsT=ones, rhs=vecsums[h][0][:],
                                     start=True, stop=False)
                    nc.tensor.matmul(den, lhsT=ones, rhs=vecsums[h][1][:],
                                     start=False, stop=True)
                else:
                    nc.tensor.matmul(den, lhsT=ones, rhs=vecsums[h][0][:],
                                     start=True, stop=True)
                recipb = spool.tile([128, TCH], f32, tag="recipb", name="recipb")
                nc.vector.reciprocal(out=recipb[:], in_=den)
                nc.vector.tensor_mul(out=yc[:, h, :], in0=ots[h], in1=recipb[:])
            return yc

        def cproj_chunk(c, yc, copy_eng=None):
            """Partial c_proj (this core's hd columns) for chunk c's rows."""
            if c == 0:
                nc.sync.dma_start(w_o[:], wo_r)
            for tt in range(TCH // 128):
                gt = c * (TCH // 128) + tt
                for nck in range(D // 512):
                    ps = mix_tile()[:, 0, :]
                    for h in range(H_LOC):
                        nc.tensor.matmul(
                            ps,
                            lhsT=yc[:, h, tt * 128:(tt + 1) * 128],
                            rhs=w_o[:, h, nck * 512:(nck + 1) * 512],
                            start=(h == 0),
                            stop=(h == H_LOC - 1),
                        )
                    ob = opool.tile([128, 512], f32, tag="ob", name="ob")
                    if copy_eng is None:
                        nc.scalar.copy(out=ob[:], in_=ps)
                    else:
                        copy_eng.tensor_copy(out=ob[:], in_=ps)
                    nc.gpsimd.dma_start(
                        out_d[gt * 128:(gt + 1) * 128,
                              nck * 512:(nck + 1) * 512],
                        ob[:],
                    )

        # Emission order: projections stream in chunk order; each attention
        # chunk is emitted as soon as its projections exist, EXCEPT chunk 0
        # (the smallest, 4 j-tiles) which is saved for the tail so the
        # ACT-bound final attention stretch is as short as possible.
        qcs = {}
        qcs[0] = proj_chunk(0)
        qcs[1] = proj_chunk(1)
        y0 = attn_chunk(0, qcs[0])
        cproj_chunk(0, y0)
        y1 = attn_chunk(1, qcs[1])
        cproj_chunk(1, y1)
        qcs[2] = proj_chunk(2)
        y2 = attn_chunk(2, qcs[2])
        cproj_chunk(2, y2)
        qcs[3] = proj_chunk(3)
        y3 = attn_chunk(3, qcs[3])
        cproj_chunk(3, y3, copy_eng=nc.vector)

    nc.compile()
    _CACHE["nc"] = nc
    return nc


def host_inputs(x, Wq, Wk, Wv, Wo):
    """Per-core input dicts (host-side shard + transpose + table prep)."""
    x2 = np.ascontiguousarray(x.reshape(T, D).T).astype(np.float32)  # (D, T)

    half = DH // 2  # 64
    af = (1.0 / 1024.0) ** np.linspace(0.0, 1.0, DH // 4, dtype=np.float32)
    af = np.concatenate([af, np.zeros(DH // 4, np.float32)])         # (64,)
    theta = np.arange(T, dtype=np.float32)[:, None] * af[None, :]    # (T, 64)
    cos = np.cos(theta).T.astype(np.float32)                         # (64, T)
    sin = np.sin(theta).T.astype(np.float32)
    ctab = np.concatenate([cos, cos], axis=0)                        # (128, T)
    stab = np.concatenate([sin, -sin], axis=0)

    roll = np.zeros((128, 128), np.float32)
    for p in range(128):
        roll[p, (p + half) % 128] = 1.0
    ones = np.ones((128, 128), np.float32)
    tri = np.triu(np.ones((128, 128), np.float32))  # tri[j, i] = i >= j

    shared = {
        "xT": x2, "ctab": ctab, "stab": stab,
        "roll": roll, "ones": ones, "tri": tri,
    }
    in_maps = []
    for c in range(N_CORES):
        sl = slice(c * HD_LOC, (c + 1) * HD_LOC)
        in_maps.append({
            **shared,
            "wqT": np.ascontiguousarray(Wq[sl, :].T),
            "wkT": np.ascontiguousarray(Wk[sl, :].T),
            "wvT": np.ascontiguousarray(Wv[sl, :].T),
            "woT": np.ascontiguousarray((Wo[:, sl] / 3.0).T),
        })
    return in_maps


def _get_runner():
    """Build the program + a persistent jitted SPMD executable (once)."""
    if "runner" in _CACHE:
        return _CACHE["runner"]

    import jax
    import concourse.mybir as mybir
    from concourse.bass2jax import (
        _bass_exec_p,
        install_neuronx_cc_hook,
        partition_id_tensor,
    )
    from jax.experimental.shard_map import shard_map
    from jax.sharding import Mesh, PartitionSpec

    nc = build_program()
    install_neuronx_cc_hook()
    assert nc.dbg_addr is None
    pid_name = nc.partition_id_tensor.name if nc.partition_id_tensor else None

    in_names, out_names, out_avals, zero_outs = [], [], [], []
    for alloc in nc.m.functions[0].allocations:
        if not isinstance(alloc, mybir.MemoryLocationSet):
            continue
        name = alloc.memorylocations[0].name
        if alloc.kind == "ExternalInput":
            if name != pid_name:
                in_names.append(name)
        elif alloc.kind == "ExternalOutput":
            out_names.append(name)
            shape = tuple(alloc.tensor_shape)
            dtype = mybir.dt.np(alloc.dtype)
            out_avals.append(jax.core.ShapedArray(shape, dtype))
            zero_outs.append(np.zeros(shape, dtype))
    n_params = len(in_names)
    all_names = list(in_names) + list(out_names)
    if pid_name is not None:
        all_names.append(pid_name)
    donate = tuple(range(n_params, n_params + len(out_names)))

    def _body(*args):
        operands = list(args)
        if pid_name is not None:
            operands.append(partition_id_tensor())
        outs = _bass_exec_p.bind(
            *operands,
            out_avals=tuple(out_avals),
            in_names=tuple(all_names),
            out_names=tuple(out_names),
            lowering_input_output_aliases=(),
            sim_require_finite=True,
            sim_require_nnan=True,
            nc=nc,
        )
        return tuple(outs)

    devices = jax.devices()[:N_CORES]
    mesh = Mesh(np.asarray(devices), ("core",))
    in_specs = (PartitionSpec("core"),) * (n_params + len(out_names))
    out_specs = (PartitionSpec("core"),) * len(out_names)
    fn = jax.jit(
        shard_map(_body, mesh=mesh, in_specs=in_specs, out_specs=out_specs,
                  check_rep=False),
        donate_argnums=donate,
        keep_unused=True,
    )
    runner = (fn, in_names, out_names, out_avals, zero_outs)
    _CACHE["runner"] = runner
    return runner


def run_spmd(in_maps):
    """Execute the SPMD program; returns per-core output dicts."""
    fn, in_names, out_names, out_avals, zero_outs = _get_runner()
    concat_in = [
        np.concatenate([np.asarray(in_maps[c][n]) for c in range(N_CORES)], axis=0)
        for n in in_names
    ]
    concat_zeros = [
        np.zeros((N_CORES * z.shape[0], *z.shape[1:]), z.dtype) for z in zero_outs
    ]
    out_arrs = fn(*concat_in, *concat_zeros)
    return [
        {n: np.asarray(out_arrs[i]).reshape(N_CORES, *out_avals[i].shape)[c]
         for i, n in enumerate(out_names)}
        for c in range(N_CORES)
    ]


def kernel(x, Wq, Wk, Wv, Wo):
    in_maps = host_inputs(np.asarray(x), np.asarray(Wq), np.asarray(Wk),
                          np.asarray(Wv), np.asarray(Wo))
    results = run_spmd(in_maps)
    out = results[0]["outp"].astype(np.float64)
    for c in range(1, N_CORES):
        out += results[c]["outp"]
    return out.astype(np.float32).reshape(1, T, D)


# revision 28
# speedup vs baseline: 152179.7397x; 1.0592x over previous
"""Causal self-attention with RoPE, tensor-parallel over heads on 8 TRN2 NeuronCores.

Model (from the reference):
    q/k/v = x @ W{q,k,v}.T          x: (1, 2048, 2048), 16 heads x 128 head_dim
    rope(q), rope(k)                half-rotation, 32 nonzero freqs
    causal softmax(q k^T / sqrt(128)) @ v
    out = (y / 3) @ Wo.T

Sharding: 2 heads per core. Each core computes its heads' q/k/v projections,
attention, and a partial c_proj (its 256 columns of the hd contraction);
the host sums the 8 partial outputs (the "all-reduce after c_proj").

Per-core kernel layout choices:
  - Everything transposed so the contraction dim is always on partitions:
    host supplies xT (D, T) plus pre-transposed weight slices.
  - Scores computed transposed (S^T[j, i] blocks) so the P @ V matmul needs
    no transposes: OT[d, i] = sum_j V[j, d]^T P^T[j, i] is produced directly
    in the layout c_proj wants.
  - Softmax without max-subtraction (scores are provably tiny: |s| < ~2),
    denominator via DVE accumulation + one all-ones matmul (broadcast sum).
  - RoPE in transposed layout via a 64-partition roll matmul + 3 DVE ops.
  - All matmuls in float32r (full PE rate at moving dim >= 256).
"""

import numpy as np

T = 2048
D = 2048
H = 16
DH = 128
N_CORES = 8
H_LOC = H // N_CORES          # heads per core = 2
HD_LOC = H_LOC * DH           # local head dims = 256
TCH = 512                     # query-chunk width
N_CH = T // TCH               # 4 chunks
KO = D // 128                 # 16 contraction subtiles
XP = 4                        # xT streamed in pieces of 4 k-subtiles
SCALE = (DH ** 0.5) / DH      # 1/sqrt(128)

_CACHE = {}


def build_program():
    """Build (once) the single-core Bass program shared by all 8 cores."""
    if "nc" in _CACHE:
        return _CACHE["nc"]

    from contextlib import ExitStack

    import concourse.bacc as bacc
    import concourse.mybir as mybir
    import concourse.tile as tile

    f32 = mybir.dt.float32
    f32r = mybir.dt.float32r
    bf16 = mybir.dt.bfloat16
    EXP = mybir.ActivationFunctionType.Exp

    nc = bacc.Bacc("TRN2", target_bir_lowering=False)

    xT_d = nc.dram_tensor("xT", (D, T), f32r, kind="ExternalInput")
    wq_d = nc.dram_tensor("wqT", (D, HD_LOC), f32r, kind="ExternalInput")
    wk_d = nc.dram_tensor("wkT", (D, HD_LOC), f32r, kind="ExternalInput")
    wv_d = nc.dram_tensor("wvT", (D, HD_LOC), f32r, kind="ExternalInput")
    wo_d = nc.dram_tensor("woT", (HD_LOC, D), f32r, kind="ExternalInput")
    ct_d = nc.dram_tensor("ctab", (128, T), f32, kind="ExternalInput")
    st_d = nc.dram_tensor("stab", (128, T), f32, kind="ExternalInput")
    roll_d = nc.dram_tensor("roll", (128, 128), f32r, kind="ExternalInput")
    ones_d = nc.dram_tensor("ones", (128, 128), f32r, kind="ExternalInput")
    tri_d = nc.dram_tensor("tri", (128, 128), f32r, kind="ExternalInput")
    out_d = nc.dram_tensor("outp", (T, D), f32, kind="ExternalOutput")

    xT_r = xT_d[:].rearrange("(ko p) t -> p ko t", p=128)
    wq_r = wq_d[:].rearrange("(ko p) m -> p ko m", p=128)
    wk_r = wk_d[:].rearrange("(ko p) m -> p ko m", p=128)
    wv_r = wv_d[:].rearrange("(ko p) m -> p ko m", p=128)
    wo_r = wo_d[:].rearrange("(h p) d -> p h d", p=128)

    with tile.TileContext(nc) as tc, ExitStack() as ctx:
        persist = ctx.enter_context(tc.tile_pool(name="persist", bufs=1))
        qpool = ctx.enter_context(tc.tile_pool(name="qpool", bufs=2))
        ypool = ctx.enter_context(tc.tile_pool(name="ypool", bufs=2))
        xpool = ctx.enter_context(tc.tile_pool(name="xpool", bufs=5))
        ptpool = ctx.enter_context(tc.tile_pool(name="ptpool", bufs=4))
        rtmp = ctx.enter_context(tc.tile_pool(name="rtmp", bufs=1))
        spool = ctx.enter_context(tc.tile_pool(name="spool", bufs=2))
        opool = ctx.enter_context(tc.tile_pool(name="opool", bufs=3))
        psum_p = ctx.enter_context(tc.tile_pool(name="psum_p", bufs=2, space="PSUM"))
        psum_mix = ctx.enter_context(tc.tile_pool(name="psum_mix", bufs=2, space="PSUM"))
        psum_ot = ctx.enter_context(tc.tile_pool(name="psum_ot", bufs=2, space="PSUM"))

        def ps_tile(pool=None):
            return (pool or psum_p).tile([128, TCH], f32, tag="ps", name="ps")

        def mix_tile():
            return psum_mix.tile([128, H_LOC, TCH], f32, tag="mix", name="mix")

        # --- resident tensors ---
        w_q = persist.tile([128, KO, HD_LOC], f32r, tag="w_q")
        w_k = persist.tile([128, KO, HD_LOC], f32r, tag="w_k")
        w_v = persist.tile([128, KO, HD_LOC], f32r, tag="w_v")
        w_o = persist.tile([128, H_LOC, D], f32r, tag="w_o")
        kt = persist.tile([128, H_LOC, T], f32r, tag="kt")
        vt = persist.tile([128, KO, HD_LOC], f32r, tag="vt")
        ctab = persist.tile([128, T], f32, tag="ctab")
        stab = persist.tile([128, T], f32, tag="stab")
        roll = persist.tile([128, 128], f32r, tag="roll")
        ones = persist.tile([128, 128], f32r, tag="ones")
        tri = persist.tile([128, 128], f32r, tag="tri")

        def proj_chunk(c):
            """q/k/v projections + RoPE for t-chunk c (xT streamed in pieces)."""
            cs = c * TCH
            pieces = []
            for kp in range(KO // XP):
                ksl = slice(kp * XP, (kp + 1) * XP)
                xc = xpool.tile([128, XP, TCH], f32r, tag="xc", name="xc")
                nc.sync.dma_start(xc[:], xT_r[:, ksl, cs:cs + TCH])
                pieces.append(xc)
                if c == 0:
                    nc.sync.dma_start(w_q[:, ksl, :], wq_r[:, ksl, :])
                    nc.sync.dma_start(w_k[:, ksl, :], wk_r[:, ksl, :])
                    nc.sync.dma_start(w_v[:, ksl, :], wv_r[:, ksl, :])
            if c == 0:
                nc.sync.dma_start(ctab[:], ct_d[:])
                nc.sync.dma_start(stab[:], st_d[:])
                nc.sync.dma_start(roll[:], roll_d[:])
                nc.sync.dma_start(ones[:], ones_d[:])
                nc.sync.dma_start(tri[:], tri_d[:])

            qc = qpool.tile([128, H_LOC, TCH], f32r, tag="qc", name="qc")
            for w_sb, dst in ((w_q, qc), (w_k, kt)):
                for h in range(H_LOC):
                    dsl = dst[:, h, :] if dst is qc else dst[:, h, cs:cs + TCH]
                    ps = ps_tile()
                    for ko in range(KO):
                        nc.tensor.matmul(
                            ps,
                            lhsT=w_sb[:, ko, h * 128:(h + 1) * 128],
                            rhs=pieces[ko // XP][:, ko % XP, :],
                            start=(ko == 0),
                            stop=(ko == KO - 1),
                        )
                    nc.scalar.copy(out=dsl, in_=ps)

            for tt in range(TCH // 128):
                gt = c * (TCH // 128) + tt
                ps = ps_tile()
                for ko in range(KO):
                    nc.tensor.matmul(
                        ps[:, :HD_LOC],
                        lhsT=pieces[ko // XP][:, ko % XP, tt * 128:(tt + 1) * 128],
                        rhs=w_v[:, ko, :],
                        start=(ko == 0),
                        stop=(ko == KO - 1),
                    )
                nc.scalar.copy(out=vt[:, gt, :], in_=ps[:, :HD_LOC])

            # RoPE: y = x*C + roll64(x)*S' (only via PE roll + 3 DVE ops)
            for srct in (qc, kt):
                for h in range(H_LOC):
                    sl = srct[:, h, :] if srct is qc else srct[:, h, cs:cs + TCH]
                    rolled = ps_tile()
                    nc.tensor.matmul(rolled, lhsT=roll, rhs=sl,
                                     start=True, stop=True)
                    a = rtmp.tile([128, TCH], f32, tag="ra", name="ra")
                    b = rtmp.tile([128, TCH], f32, tag="rb", name="rb")
                    nc.vector.tensor_mul(out=a, in0=sl, in1=ctab[:, cs:cs + TCH])
                    nc.vector.tensor_mul(out=b, in0=rolled, in1=stab[:, cs:cs + TCH])
                    nc.vector.tensor_add(out=sl, in0=a, in1=b)
            return qc

        def attn_chunk(c, qc):
            """Causal attention for query chunk c, heads interleaved."""
            cs = c * TCH
            yc = ypool.tile([128, H_LOC, TCH], f32r, tag="yc", name="yc")
            n_jt = 4 * c + 4
            ots = [ps_tile(psum_ot) for _ in range(H_LOC)]
            vecsums = [[spool.tile([128, TCH], f32r, tag=f"vecsum{par}",
                                   name="vecsum")
                        for par in range(2)] for _ in range(H_LOC)]
            for jt in range(n_jt):
                pair = mix_tile()
                for h in range(H_LOC):
                    nc.tensor.matmul(
                        pair[:, h, :],
                        lhsT=kt[:, h, jt * 128:(jt + 1) * 128],
                        rhs=qc[:, h, :],
                        start=True,
                        stop=True,
                    )
                pt = ptpool.tile([128, H_LOC, TCH], f32r, tag="pt", name="pt")
                m = jt - 4 * c
                # diagonal block: cols < 128m fully masked -- never written,
                # never read (partial-width ops)
                lo = 128 * m if m > 0 else 0
                if lo == 0:
                    # both heads in ONE activation call (contiguous 1024 wide)
                    nc.scalar.activation(out=pt[:, :, :], in_=pair[:, :, :],
                                         func=EXP, scale=SCALE)
                else:
                    for h in range(H_LOC):
                        nc.scalar.activation(out=pt[:, h, lo:],
                                             in_=pair[:, h, lo:],
                                             func=EXP, scale=SCALE)
                for h in range(H_LOC):
                    if m >= 0:
                        nc.vector.tensor_mul(
                            out=pt[:, h, 128 * m:128 * (m + 1)],
                            in0=pt[:, h, 128 * m:128 * (m + 1)],
                            in1=tri[:],
                        )
                    # chunk 0: jt==1 is diagonal (cols < 128 unwritten), so a
                    # full-width init copy would ingest garbage -- use a single
                    # DVE accumulator there. Other chunks split DVE/GPSIMD.
                    par = jt % 2 if c > 0 else 0
                    vs = vecsums[h][par]
                    eng = nc.vector if par == 0 else nc.gpsimd
                    if jt < (2 if c > 0 else 1):
                        eng.tensor_copy(out=vs[:], in_=pt[:, h, :])
                    else:
                        eng.tensor_add(out=vs[:, lo:], in0=vs[:, lo:],
                                       in1=pt[:, h, lo:])
                    nc.tensor.matmul(
                        ots[h][:, lo:],
                        lhsT=vt[:, jt, h * 128:(h + 1) * 128],
                        rhs=pt[:, h, lo:],
                        start=(jt == 0),
                        stop=(jt == n_jt - 1),
                        skip_group_check=(lo > 0),
                    )
            for h in range(H_LOC):
                # denominator: all-ones matmul -> column sums on all partitions
                den = mix_tile()[:, 0, :]
                if c > 0:
                    nc.tensor.matmul(den, lhsT=ones, rhs=vecsums[h][0][:],
                                     start=True, stop=False)
                    nc.tensor.matmul(den, lhsT=ones, rhs=vecsums[h][1][:],
                                     start=False, stop=True)
                else:
                    nc.tensor.matmul(den, lhsT=ones, rhs=vecsums[h][0][:],
                                     start=True, stop=True)
                recipb = spool.tile([128, TCH], f32, tag="recipb", name="recipb")
                nc.vector.reciprocal(out=recipb[:], in_=den)
                nc.vector.tensor_mul(out=yc[:, h, :], in0=ots[h], in1=recipb[:])
            return yc

        def cproj_chunk(c, yc, copy_eng=None):
            """Partial c_proj (this core's hd columns) for chunk c's rows."""
            if c == 0:
                nc.sync.dma_start(w_o[:], wo_r)
            for tt in range(TCH // 128):
                gt = c * (TCH // 128) + tt
                for nck in range(D // 512):
                    ps = mix_tile()[:, 0, :]
                    for h in range(H_LOC):
                        nc.tensor.matmul(
                            ps,
                            lhsT=yc[:, h, tt * 128:(tt + 1) * 128],
                            rhs=w_o[:, h, nck * 512:(nck + 1) * 512],
                            start=(h == 0),
                            stop=(h == H_LOC - 1),
                        )
                    ob = opool.tile([128, 512], f32, tag="ob", name="ob")
                    if copy_eng is None:
                        nc.scalar.copy(out=ob[:], in_=ps)
                    else:
                        copy_eng.tensor_copy(out=ob[:], in_=ps)
                    nc.sync.dma_start(
                        out_d[gt * 128:(gt + 1) * 128,
                              nck * 512:(nck + 1) * 512],
                        ob[:],
                    )

        # Emission order: projections stream in chunk order; each attention
        # chunk is emitted as soon as its projections exist, EXCEPT chunk 0
        # (the smallest, 4 j-tiles) which is saved for the tail so the
        # ACT-bound final attention stretch is as short as possible.
        for c in range(N_CH):
            qc = proj_chunk(c)
            yc = attn_chunk(c, qc)
            cproj_chunk(c, yc, copy_eng=nc.vector if c == N_CH - 1 else None)

    nc.compile()
    _CACHE["nc"] = nc
    return nc


def host_inputs(x, Wq, Wk, Wv, Wo):
    """Per-core input dicts (host-side shard + transpose + table prep)."""
    x2 = np.ascontiguousarray(x.reshape(T, D).T).astype(np.float32)  # (D, T)

    half = DH // 2  # 64
    af = (1.0 / 1024.0) ** np.linspace(0.0, 1.0, DH // 4, dtype=np.float32)
    af = np.concatenate([af, np.zeros(DH // 4, np.float32)])         # (64,)
    theta = np.arange(T, dtype=np.float32)[:, None] * af[None, :]    # (T, 64)
    cos = np.cos(theta).T.astype(np.float32)                         # (64, T)
    sin = np.sin(theta).T.astype(np.float32)
    ctab = np.concatenate([cos, cos], axis=0)                        # (128, T)
    stab = np.concatenate([sin, -sin], axis=0)

    roll = np.zeros((128, 128), np.float32)
    for p in range(128):
        roll[p, (p + half) % 128] = 1.0
    ones = np.ones((128, 128), np.float32)
    tri = np.triu(np.ones((128, 128), np.float32))  # tri[j, i] = i >= j

    shared = {
        "xT": x2, "ctab": ctab, "stab": stab,
        "roll": roll, "ones": ones, "tri": tri,
    }
    in_maps = []
    for c in range(N_CORES):
        sl = slice(c * HD_LOC, (c + 1) * HD_LOC)
        in_maps.append({
            **shared,
            "wqT": np.ascontiguousarray(Wq[sl, :].T),
            "wkT": np.ascontiguousarray(Wk[sl, :].T),
            "wvT": np.ascontiguousarray(Wv[sl, :].T),
            "woT": np.ascontiguousarray((Wo[:, sl] / 3.0).T),
        })
    return in_maps


def _get_runner():
    """Build the program + a persistent jitted SPMD executable (once)."""
    if "runner" in _CACHE:
        return _CACHE["runner"]

    import jax
    import concourse.mybir as mybir
    from concourse.bass2jax import (
        _bass_exec_p,
        install_neuronx_cc_hook,
        partition_id_tensor,
    )
    from jax.experimental.shard_map import shard_map
    from jax.sharding import Mesh, PartitionSpec

    nc = build_program()
    install_neuronx_cc_hook()
    assert nc.dbg_addr is None
    pid_name = nc.partition_id_tensor.name if nc.partition_id_tensor else None

    in_names, out_names, out_avals, zero_outs = [], [], [], []
    for alloc in nc.m.functions[0].allocations:
        if not isinstance(alloc, mybir.MemoryLocationSet):
            continue
        name = alloc.memorylocations[0].name
        if alloc.kind == "ExternalInput":
            if name != pid_name:
                in_names.append(name)
        elif alloc.kind == "ExternalOutput":
            out_names.append(name)
            shape = tuple(alloc.tensor_shape)
            dtype = mybir.dt.np(alloc.dtype)
            out_avals.append(jax.core.ShapedArray(shape, dtype))
            zero_outs.append(np.zeros(shape, dtype))
    n_params = len(in_names)
    all_names = list(in_names) + list(out_names)
    if pid_name is not None:
        all_names.append(pid_name)
    donate = tuple(range(n_params, n_params + len(out_names)))

    def _body(*args):
        operands = list(args)
        if pid_name is not None:
            operands.append(partition_id_tensor())
        outs = _bass_exec_p.bind(
            *operands,
            out_avals=tuple(out_avals),
            in_names=tuple(all_names),
            out_names=tuple(out_names),
            lowering_input_output_aliases=(),
            sim_require_finite=True,
            sim_require_nnan=True,
            nc=nc,
        )
        return tuple(outs)

    devices = jax.devices()[:N_CORES]
    mesh = Mesh(np.asarray(devices), ("core",))
    in_specs = (PartitionSpec("core"),) * (n_params + len(out_names))
    out_specs = (PartitionSpec("core"),) * len(out_names)
    fn = jax.jit(
        shard_map(_body, mesh=mesh, in_specs=in_specs, out_specs=out_specs,
                  check_rep=False),
        donate_argnums=donate,
        keep_unused=True,
    )
    runner = (fn, in_names, out_names, out_avals, zero_outs)
    _CACHE["runner"] = runner
    return runner


def run_spmd(in_maps):
    """Execute the SPMD program; returns per-core output dicts."""
    fn, in_names, out_names, out_avals, zero_outs = _get_runner()
    concat_in = [
        np.concatenate([np.asarray(in_maps[c][n]) for c in range(N_CORES)], axis=0)
        for n in in_names
    ]
    concat_zeros = [
        np.zeros((N_CORES * z.shape[0], *z.shape[1:]), z.dtype) for z in zero_outs
    ]
    out_arrs = fn(*concat_in, *concat_zeros)
    return [
        {n: np.asarray(out_arrs[i]).reshape(N_CORES, *out_avals[i].shape)[c]
         for i, n in enumerate(out_names)}
        for c in range(N_CORES)
    ]


def kernel(x, Wq, Wk, Wv, Wo):
    in_maps = host_inputs(np.asarray(x), np.asarray(Wq), np.asarray(Wk),
                          np.asarray(Wv), np.asarray(Wo))
    results = run_spmd(in_maps)
    out = results[0]["outp"].astype(np.float64)
    for c in range(1, N_CORES):
        out += results[c]["outp"]
    return out.astype(np.float32).reshape(1, T, D)


# revision 41
# speedup vs baseline: 158574.7414x; 1.0420x over previous
"""Causal self-attention with RoPE, tensor-parallel over heads on 8 TRN2 NeuronCores.

Model (from the reference):
    q/k/v = x @ W{q,k,v}.T          x: (1, 2048, 2048), 16 heads x 128 head_dim
    rope(q), rope(k)                half-rotation, 32 nonzero freqs
    causal softmax(q k^T / sqrt(128)) @ v
    out = (y / 3) @ Wo.T

Sharding: 2 heads per core. Each core computes its heads' q/k/v projections,
attention, and a partial c_proj (its 256 columns of the hd contraction);
the host sums the 8 partial outputs (the "all-reduce after c_proj").

Per-core kernel layout choices:
  - Everything transposed so the contraction dim is always on partitions:
    host supplies xT (D, T) plus pre-transposed weight slices.
  - Scores computed transposed (S^T[j, i] blocks) so the P @ V matmul needs
    no transposes: OT[d, i] = sum_j V[j, d]^T P^T[j, i] is produced directly
    in the layout c_proj wants.
  - Softmax without max-subtraction (scores are provably tiny: |s| < ~2),
    denominator via DVE accumulation + one all-ones matmul (broadcast sum).
  - RoPE in transposed layout via a 64-partition roll matmul + 3 DVE ops.
  - All matmuls in float32r (full PE rate at moving dim >= 256).
"""

import numpy as np

T = 2048
D = 2048
H = 16
DH = 128
N_CORES = 8
H_LOC = H // N_CORES          # heads per core = 2
HD_LOC = H_LOC * DH           # local head dims = 256
TCH = 512                     # query-chunk width
N_CH = T // TCH               # 4 chunks
KO = D // 128                 # 16 contraction subtiles
XP = 4                        # xT streamed in pieces of 4 k-subtiles
SCALE = (DH ** 0.5) / DH      # 1/sqrt(128)

_CACHE = {}


def build_program():
    """Build (once) the single-core Bass program shared by all 8 cores."""
    if "nc" in _CACHE:
        return _CACHE["nc"]

    from contextlib import ExitStack

    import concourse.bacc as bacc
    import concourse.mybir as mybir
    import concourse.tile as tile

    f32 = mybir.dt.float32
    f32r = mybir.dt.float32r
    bf16 = mybir.dt.bfloat16
    EXP = mybir.ActivationFunctionType.Exp

    nc = bacc.Bacc("TRN2", target_bir_lowering=False)

    xT_d = nc.dram_tensor("xT", (D, T), f32r, kind="ExternalInput")
    wq_d = nc.dram_tensor("wqT", (D, HD_LOC), f32r, kind="ExternalInput")
    wk_d = nc.dram_tensor("wkT", (D, HD_LOC), f32r, kind="ExternalInput")
    wv_d = nc.dram_tensor("wvT", (D, HD_LOC), f32r, kind="ExternalInput")
    wo_d = nc.dram_tensor("woT", (HD_LOC, D), f32r, kind="ExternalInput")
    ct_d = nc.dram_tensor("ctab", (128, T), f32, kind="ExternalInput")
    st_d = nc.dram_tensor("stab", (128, T), f32, kind="ExternalInput")
    roll_d = nc.dram_tensor("roll", (128, 128), f32r, kind="ExternalInput")
    ones_d = nc.dram_tensor("ones", (128, 128), f32r, kind="ExternalInput")
    tri_d = nc.dram_tensor("tri", (128, 128), f32r, kind="ExternalInput")
    out_d = nc.dram_tensor("outp", (T, D), f32, kind="ExternalOutput")

    xT_r = xT_d[:].rearrange("(ko p) t -> p ko t", p=128)
    wq_r = wq_d[:].rearrange("(ko p) m -> p ko m", p=128)
    wk_r = wk_d[:].rearrange("(ko p) m -> p ko m", p=128)
    wv_r = wv_d[:].rearrange("(ko p) m -> p ko m", p=128)
    wo_r = wo_d[:].rearrange("(h p) d -> p h d", p=128)

    with tile.TileContext(nc) as tc, ExitStack() as ctx:
        persist = ctx.enter_context(tc.tile_pool(name="persist", bufs=1))
        qpool = ctx.enter_context(tc.tile_pool(name="qpool", bufs=2))
        ypool = ctx.enter_context(tc.tile_pool(name="ypool", bufs=2))
        xpool = ctx.enter_context(tc.tile_pool(name="xpool", bufs=5))
        ptpool = ctx.enter_context(tc.tile_pool(name="ptpool", bufs=4))
        rtmp = ctx.enter_context(tc.tile_pool(name="rtmp", bufs=1))
        spool = ctx.enter_context(tc.tile_pool(name="spool", bufs=2))
        opool = ctx.enter_context(tc.tile_pool(name="opool", bufs=3))
        psum_p = ctx.enter_context(tc.tile_pool(name="psum_p", bufs=2, space="PSUM"))
        psum_mix = ctx.enter_context(tc.tile_pool(name="psum_mix", bufs=2, space="PSUM"))
        psum_ot = ctx.enter_context(tc.tile_pool(name="psum_ot", bufs=2, space="PSUM"))

        def ps_tile(pool=None):
            return (pool or psum_p).tile([128, TCH], f32, tag="ps", name="ps")

        def mix_tile():
            return psum_mix.tile([128, H_LOC, TCH], f32, tag="mix", name="mix")

        # --- resident tensors ---
        w_q = persist.tile([128, KO, HD_LOC], f32r, tag="w_q")
        w_k = persist.tile([128, KO, HD_LOC], f32r, tag="w_k")
        w_v = persist.tile([128, KO, HD_LOC], f32r, tag="w_v")
        w_o = persist.tile([128, H_LOC, D], f32r, tag="w_o")
        kt = persist.tile([128, H_LOC, T], f32r, tag="kt")
        vt = persist.tile([128, KO, HD_LOC], f32r, tag="vt")
        ctab = persist.tile([128, T], f32, tag="ctab")
        stab = persist.tile([128, T], f32, tag="stab")
        roll = persist.tile([128, 128], f32r, tag="roll")
        ones = persist.tile([128, 128], f32r, tag="ones")
        tri = persist.tile([128, 128], f32r, tag="tri")

        def proj_chunk(c):
            """q/k/v projections + RoPE for t-chunk c (xT streamed in pieces)."""
            cs = c * TCH
            pieces = []
            for kp in range(KO // XP):
                ksl = slice(kp * XP, (kp + 1) * XP)
                xc = xpool.tile([128, XP, TCH], f32r, tag="xc", name="xc")
                nc.sync.dma_start(xc[:], xT_r[:, ksl, cs:cs + TCH])
                pieces.append(xc)
                if c == 0:
                    nc.sync.dma_start(w_q[:, ksl, :], wq_r[:, ksl, :])
                    nc.sync.dma_start(w_k[:, ksl, :], wk_r[:, ksl, :])
                    nc.sync.dma_start(w_v[:, ksl, :], wv_r[:, ksl, :])
            if c == 0:
                nc.sync.dma_start(ctab[:], ct_d[:])
                nc.sync.dma_start(stab[:], st_d[:])
                nc.sync.dma_start(roll[:], roll_d[:])
                nc.sync.dma_start(ones[:], ones_d[:])
                nc.sync.dma_start(tri[:], tri_d[:])

            qc = qpool.tile([128, H_LOC, TCH], f32r, tag="qc", name="qc")
            for w_sb, dst in ((w_q, qc), (w_k, kt)):
                for h in range(H_LOC):
                    dsl = dst[:, h, :] if dst is qc else dst[:, h, cs:cs + TCH]
                    # chunk 0 is DMA-paced: borrow the idle attention pools so
                    # all 8 projection groups can accumulate concurrently and
                    # every arriving piece has ready matmuls
                    ps = ps_tile(psum_ot if (c == 0 and dst is kt) else None)
                    for ko in range(KO):
                        nc.tensor.matmul(
                            ps,
                            lhsT=w_sb[:, ko, h * 128:(h + 1) * 128],
                            rhs=pieces[ko // XP][:, ko % XP, :],
                            start=(ko == 0),
                            stop=(ko == KO - 1),
                        )
                    nc.scalar.copy(out=dsl, in_=ps)

            # RoPE: y = x*C + roll64(x)*S' (only via PE roll + 3 DVE ops)
            for srct in (qc, kt):
                for h in range(H_LOC):
                    sl = srct[:, h, :] if srct is qc else srct[:, h, cs:cs + TCH]
                    rolled = ps_tile()
                    nc.tensor.matmul(rolled, lhsT=roll, rhs=sl,
                                     start=True, stop=True)
                    a = rtmp.tile([128, TCH], f32, tag="ra", name="ra")
                    b = rtmp.tile([128, TCH], f32, tag="rb", name="rb")
                    nc.vector.tensor_mul(out=a, in0=sl, in1=ctab[:, cs:cs + TCH])
                    nc.vector.tensor_mul(out=b, in0=rolled, in1=stab[:, cs:cs + TCH])
                    nc.vector.tensor_add(out=sl, in0=a, in1=b)
            vmix = mix_tile()
            for tt in range(TCH // 128):
                gt = c * (TCH // 128) + tt
                ps = vmix[:, tt // 2, (tt % 2) * HD_LOC:(tt % 2 + 1) * HD_LOC]
                for ko in range(KO):
                    nc.tensor.matmul(
                        ps[:, :HD_LOC],
                        lhsT=pieces[ko // XP][:, ko % XP, tt * 128:(tt + 1) * 128],
                        rhs=w_v[:, ko, :],
                        start=(ko == 0),
                        stop=(ko == KO - 1),
                    )
                nc.scalar.copy(out=vt[:, gt, :], in_=ps[:, :HD_LOC])

            return qc

        def attn_span(q0, W, qc, off, yc):
            """Causal attention for queries [q0, q0+W), heads interleaved.

            q0 must be 128-aligned; W in {256, 512}. qc holds the chunk's
            roped queries; off is q0's offset within qc/yc."""
            d0 = q0 // 128          # first diagonal j-tile
            n_jt = d0 + W // 128
            ots = [ps_tile(psum_ot) for _ in range(H_LOC)]
            vecsums = [[spool.tile([128, TCH], f32r, tag=f"vecsum{par}",
                                   name="vecsum")
                        for par in range(2)] for _ in range(H_LOC)]
            for jt in range(n_jt):
                pair = mix_tile()
                for h in range(H_LOC):
                    nc.tensor.matmul(
                        pair[:, h, :W],
                        lhsT=kt[:, h, jt * 128:(jt + 1) * 128],
                        rhs=qc[:, h, off:off + W],
                        start=True,
                        stop=True,
                    )
                pt = ptpool.tile([128, H_LOC, TCH], f32r, tag="pt", name="pt")
                m = jt - d0
                # diagonal block: cols < 128m fully masked -- never written,
                # never read (partial-width ops)
                lo = 128 * m if m > 0 else 0
                # both heads in ONE activation call (strided AP when lo > 0)
                nc.scalar.activation(out=pt[:, :, lo:W], in_=pair[:, :, lo:W],
                                     func=EXP, scale=SCALE)
                for h in range(H_LOC):
                    if m >= 0:
                        nc.vector.tensor_mul(
                            out=pt[:, h, 128 * m:128 * (m + 1)],
                            in0=pt[:, h, 128 * m:128 * (m + 1)],
                            in1=tri[:],
                        )
                    # spans starting at q0=0: jt==1 is diagonal (cols < 128
                    # unwritten), so a full-width init copy would ingest
                    # garbage -- single DVE accumulator there. Other spans
                    # split across DVE (even jt) and GPSIMD (odd jt).
                    par = jt % 2 if d0 >= 2 else 0
                    vs = vecsums[h][par]
                    eng = nc.vector if par == 0 else nc.gpsimd
                    if jt < (2 if d0 >= 2 else 1):
                        eng.tensor_copy(out=vs[:, :W], in_=pt[:, h, :W])
                    else:
                        eng.tensor_add(out=vs[:, lo:W], in0=vs[:, lo:W],
                                       in1=pt[:, h, lo:W])
                    nc.tensor.matmul(
                        ots[h][:, lo:W],
                        lhsT=vt[:, jt, h * 128:(h + 1) * 128],
                        rhs=pt[:, h, lo:W],
                        start=(jt == 0),
                        stop=(jt == n_jt - 1),
                        skip_group_check=(lo > 0),
                    )
            for h in range(H_LOC):
                # denominator: all-ones matmul -> column sums on all partitions
                den = mix_tile()[:, 0, :W]
                if d0 >= 2:
                    nc.tensor.matmul(den, lhsT=ones, rhs=vecsums[h][0][:, :W],
                                     start=True, stop=False)
                    nc.tensor.matmul(den, lhsT=ones, rhs=vecsums[h][1][:, :W],
                                     start=False, stop=True)
                else:
                    nc.tensor.matmul(den, lhsT=ones, rhs=vecsums[h][0][:, :W],
                                     start=True, stop=True)
                recipb = spool.tile([128, TCH], f32, tag="recipb", name="recipb")
                nc.vector.reciprocal(out=recipb[:, :W], in_=den)
                nc.vector.tensor_mul(out=yc[:, h, off:off + W],
                                     in0=ots[h][:, :W], in1=recipb[:, :W])

        def cproj_span(q0, W, yc, off, copy_eng=None):
            """Partial c_proj (this core's hd columns) for rows [q0, q0+W)."""
            if q0 == 0:
                nc.sync.dma_start(w_o[:], wo_r)
            for tt in range(W // 128):
                gt = q0 // 128 + tt
                for nck in range(D // 512):
                    ps = mix_tile()[:, 0, :]
                    for h in range(H_LOC):
                        nc.tensor.matmul(
                            ps,
                            lhsT=yc[:, h, off + tt * 128:off + (tt + 1) * 128],
                            rhs=w_o[:, h, nck * 512:(nck + 1) * 512],
                            start=(h == 0),
                            stop=(h == H_LOC - 1),
                        )
                    ob = opool.tile([128, 512], f32, tag="ob", name="ob")
                    if copy_eng is None:
                        nc.scalar.copy(out=ob[:], in_=ps)
                    else:
                        copy_eng.tensor_copy(out=ob[:], in_=ps)
                    nc.sync.dma_start(
                        out_d[gt * 128:(gt + 1) * 128,
                              nck * 512:(nck + 1) * 512],
                        ob[:],
                    )

        # Emission order: projections stream in chunk order; each attention
        # chunk is emitted as soon as its projections exist, EXCEPT chunk 0
        # (the smallest, 4 j-tiles) which is saved for the tail so the
        # ACT-bound final attention stretch is as short as possible.
        for c in range(N_CH - 1):
            qc = proj_chunk(c)
            yc = ypool.tile([128, H_LOC, TCH], f32r, tag="yc", name="yc")
            attn_span(c * TCH, TCH, qc, 0, yc)
            cproj_span(c * TCH, TCH, yc, 0)
        c = N_CH - 1
        qc = proj_chunk(c)
        yc = ypool.tile([128, H_LOC, TCH], f32r, tag="yc", name="yc")
        attn_span(c * TCH, TCH, qc, 0, yc)
        cproj_span(c * TCH, TCH, yc, 0)

    nc.compile()
    _CACHE["nc"] = nc
    return nc


def host_inputs(x, Wq, Wk, Wv, Wo):
    """Per-core input dicts (host-side shard + transpose + table prep)."""
    x2 = np.ascontiguousarray(x.reshape(T, D).T).astype(np.float32)  # (D, T)

    half = DH // 2  # 64
    af = (1.0 / 1024.0) ** np.linspace(0.0, 1.0, DH // 4, dtype=np.float32)
    af = np.concatenate([af, np.zeros(DH // 4, np.float32)])         # (64,)
    theta = np.arange(T, dtype=np.float32)[:, None] * af[None, :]    # (T, 64)
    cos = np.cos(theta).T.astype(np.float32)                         # (64, T)
    sin = np.sin(theta).T.astype(np.float32)
    ctab = np.concatenate([cos, cos], axis=0)                        # (128, T)
    stab = np.concatenate([sin, -sin], axis=0)

    roll = np.zeros((128, 128), np.float32)
    for p in range(128):
        roll[p, (p + half) % 128] = 1.0
    ones = np.ones((128, 128), np.float32)
    tri = np.triu(np.ones((128, 128), np.float32))  # tri[j, i] = i >= j

    shared = {
        "xT": x2, "ctab": ctab, "stab": stab,
        "roll": roll, "ones": ones, "tri": tri,
    }
    in_maps = []
    for c in range(N_CORES):
        sl = slice(c * HD_LOC, (c + 1) * HD_LOC)
        in_maps.append({
            **shared,
            "wqT": np.ascontiguousarray(Wq[sl, :].T),
            "wkT": np.ascontiguousarray(Wk[sl, :].T),
            "wvT": np.ascontiguousarray(Wv[sl, :].T),
            "woT": np.ascontiguousarray((Wo[:, sl] / 3.0).T),
        })
    return in_maps


def _get_runner():
    """Build the program + a persistent jitted SPMD executable (once)."""
    if "runner" in _CACHE:
        return _CACHE["runner"]

    import jax
    import concourse.mybir as mybir
    from concourse.bass2jax import (
        _bass_exec_p,
        install_neuronx_cc_hook,
        partition_id_tensor,
    )
    from jax.experimental.shard_map import shard_map
    from jax.sharding import Mesh, PartitionSpec

    nc = build_program()
    install_neuronx_cc_hook()
    assert nc.dbg_addr is None
    pid_name = nc.partition_id_tensor.name if nc.partition_id_tensor else None

    in_names, out_names, out_avals, zero_outs = [], [], [], []
    for alloc in nc.m.functions[0].allocations:
        if not isinstance(alloc, mybir.MemoryLocationSet):
            continue
        name = alloc.memorylocations[0].name
        if alloc.kind == "ExternalInput":
            if name != pid_name:
                in_names.append(name)
        elif alloc.kind == "ExternalOutput":
            out_names.append(name)
            shape = tuple(alloc.tensor_shape)
            dtype = mybir.dt.np(alloc.dtype)
            out_avals.append(jax.core.ShapedArray(shape, dtype))
            zero_outs.append(np.zeros(shape, dtype))
    n_params = len(in_names)
    all_names = list(in_names) + list(out_names)
    if pid_name is not None:
        all_names.append(pid_name)
    donate = tuple(range(n_params, n_params + len(out_names)))

    def _body(*args):
        operands = list(args)
        if pid_name is not None:
            operands.append(partition_id_tensor())
        outs = _bass_exec_p.bind(
            *operands,
            out_avals=tuple(out_avals),
            in_names=tuple(all_names),
            out_names=tuple(out_names),
            lowering_input_output_aliases=(),
            sim_require_finite=True,
            sim_require_nnan=True,
            nc=nc,
        )
        return tuple(outs)

    devices = jax.devices()[:N_CORES]
    mesh = Mesh(np.asarray(devices), ("core",))
    in_specs = (PartitionSpec("core"),) * (n_params + len(out_names))
    out_specs = (PartitionSpec("core"),) * len(out_names)
    fn = jax.jit(
        shard_map(_body, mesh=mesh, in_specs=in_specs, out_specs=out_specs,
                  check_rep=False),
        donate_argnums=donate,
        keep_unused=True,
    )
    runner = (fn, in_names, out_names, out_avals, zero_outs)
    _CACHE["runner"] = runner
    return runner


def run_spmd(in_maps):
    """Execute the SPMD program; returns per-core output dicts."""
    fn, in_names, out_names, out_avals, zero_outs = _get_runner()
    concat_in = [
        np.concatenate([np.asarray(in_maps[c][n]) for c in range(N_CORES)], axis=0)
        for n in in_names
    ]
    concat_zeros = [
        np.zeros((N_CORES * z.shape[0], *z.shape[1:]), z.dtype) for z in zero_outs
    ]
    out_arrs = fn(*concat_in, *concat_zeros)
    return [
        {n: np.asarray(out_arrs[i]).reshape(N_CORES, *out_avals[i].shape)[c]
         for i, n in enumerate(out_names)}
        for c in range(N_CORES)
    ]


def kernel(x, Wq, Wk, Wv, Wo):
    in_maps = host_inputs(np.asarray(x), np.asarray(Wq), np.asarray(Wk),
                          np.asarray(Wv), np.asarray(Wo))
    results = run_spmd(in_maps)
    out = results[0]["outp"].astype(np.float64)
    for c in range(1, N_CORES):
        out += results[c]["outp"]
    return out.astype(np.float32).reshape(1, T, D)


# revision 44
# speedup vs baseline: 171428.3442x; 1.0811x over previous
"""Causal self-attention with RoPE, tensor-parallel over heads on 8 TRN2 NeuronCores.

Model (from the reference):
    q/k/v = x @ W{q,k,v}.T          x: (1, 2048, 2048), 16 heads x 128 head_dim
    rope(q), rope(k)                half-rotation, 32 nonzero freqs
    causal softmax(q k^T / sqrt(128)) @ v
    out = (y / 3) @ Wo.T

Sharding: 2 heads per core. Each core computes its heads' q/k/v projections,
attention, and a partial c_proj (its 256 columns of the hd contraction);
the host sums the 8 partial outputs (the "all-reduce after c_proj").

Per-core kernel layout choices:
  - Everything transposed so the contraction dim is always on partitions:
    host supplies xT (D, T) plus pre-transposed weight slices.
  - Scores computed transposed (S^T[j, i] blocks) so the P @ V matmul needs
    no transposes: OT[d, i] = sum_j V[j, d]^T P^T[j, i] is produced directly
    in the layout c_proj wants.
  - Softmax without max-subtraction (scores are provably tiny: |s| < ~2),
    denominator via DVE accumulation + one all-ones matmul (broadcast sum).
  - RoPE in transposed layout via a 64-partition roll matmul + 3 DVE ops.
  - All matmuls in float32r (full PE rate at moving dim >= 256).
"""

import numpy as np

T = 2048
D = 2048
H = 16
DH = 128
N_CORES = 8
H_LOC = H // N_CORES          # heads per core = 2
HD_LOC = H_LOC * DH           # local head dims = 256
TCH = 512                     # query-chunk width
N_CH = T // TCH               # 4 chunks
KO = D // 128                 # 16 contraction subtiles
XP = 4                        # xT streamed in pieces of 4 k-subtiles
SCALE = (DH ** 0.5) / DH      # 1/sqrt(128)

_CACHE = {}


def build_program():
    """Build (once) the single-core Bass program shared by all 8 cores."""
    if "nc" in _CACHE:
        return _CACHE["nc"]

    from contextlib import ExitStack

    import concourse.bacc as bacc
    import concourse.mybir as mybir
    import concourse.tile as tile

    f32 = mybir.dt.float32
    f32r = mybir.dt.float32r
    bf16 = mybir.dt.bfloat16
    EXP = mybir.ActivationFunctionType.Exp

    nc = bacc.Bacc("TRN2", target_bir_lowering=False)

    xT_d = nc.dram_tensor("xT", (D, T), f32r, kind="ExternalInput")
    wq_d = nc.dram_tensor("wqT", (D, HD_LOC), f32r, kind="ExternalInput")
    wk_d = nc.dram_tensor("wkT", (D, HD_LOC), f32r, kind="ExternalInput")
    wv_d = nc.dram_tensor("wvT", (D, HD_LOC), f32r, kind="ExternalInput")
    wo_d = nc.dram_tensor("woT", (HD_LOC, D), f32r, kind="ExternalInput")
    ct_d = nc.dram_tensor("ctab", (128, T), f32, kind="ExternalInput")
    st_d = nc.dram_tensor("stab", (128, T), f32, kind="ExternalInput")
    roll_d = nc.dram_tensor("roll", (128, 128), f32r, kind="ExternalInput")
    ones_d = nc.dram_tensor("ones", (128, 128), f32r, kind="ExternalInput")
    tri_d = nc.dram_tensor("tri", (128, 128), f32r, kind="ExternalInput")
    out_d = nc.dram_tensor("outp", (T, D), f32, kind="ExternalOutput")

    xT_r = xT_d[:].rearrange("(ko p) t -> p ko t", p=128)
    wq_r = wq_d[:].rearrange("(ko p) m -> p ko m", p=128)
    wk_r = wk_d[:].rearrange("(ko p) m -> p ko m", p=128)
    wv_r = wv_d[:].rearrange("(ko p) m -> p ko m", p=128)
    wo_r = wo_d[:].rearrange("(h p) d -> p h d", p=128)

    with tile.TileContext(nc) as tc, ExitStack() as ctx:
        persist = ctx.enter_context(tc.tile_pool(name="persist", bufs=1))
        qpool = ctx.enter_context(tc.tile_pool(name="qpool", bufs=2))
        ypool = ctx.enter_context(tc.tile_pool(name="ypool", bufs=2))
        xpool = ctx.enter_context(tc.tile_pool(name="xpool", bufs=5))
        ptpool = ctx.enter_context(tc.tile_pool(name="ptpool", bufs=3))
        rtmp = ctx.enter_context(tc.tile_pool(name="rtmp", bufs=1))
        spool = ctx.enter_context(tc.tile_pool(name="spool", bufs=2))
        opool = ctx.enter_context(tc.tile_pool(name="opool", bufs=6))
        psum_p = ctx.enter_context(tc.tile_pool(name="psum_p", bufs=2, space="PSUM"))
        psum_mix = ctx.enter_context(tc.tile_pool(name="psum_mix", bufs=2, space="PSUM"))
        psum_ot = ctx.enter_context(tc.tile_pool(name="psum_ot", bufs=2, space="PSUM"))

        def ps_tile(pool=None):
            return (pool or psum_p).tile([128, TCH], f32, tag="ps", name="ps")

        def mix_tile():
            return psum_mix.tile([128, H_LOC, TCH], f32, tag="mix", name="mix")

        # --- resident tensors ---
        w_q = persist.tile([128, KO, HD_LOC], f32r, tag="w_q")
        w_k = persist.tile([128, KO, HD_LOC], f32r, tag="w_k")
        w_v = persist.tile([128, KO, HD_LOC], f32r, tag="w_v")
        w_o = persist.tile([128, H_LOC, D], f32r, tag="w_o")
        kt = persist.tile([128, H_LOC, T], f32r, tag="kt")
        vt = persist.tile([128, KO, HD_LOC], f32r, tag="vt")
        ctab = persist.tile([128, T], f32, tag="ctab")
        stab = persist.tile([128, T], f32, tag="stab")
        roll = persist.tile([128, 128], f32r, tag="roll")
        ones = persist.tile([128, 128], f32r, tag="ones")
        tri = persist.tile([128, 128], f32r, tag="tri")

        def issue_x(c):
            """Queue the xT piece DMAs for chunk c (weights too on chunk 0)."""
            cs = c * TCH
            pieces = []
            for kp in range(KO // XP):
                ksl = slice(kp * XP, (kp + 1) * XP)
                xc = xpool.tile([128, XP, TCH], f32r, tag="xc", name="xc")
                nc.sync.dma_start(xc[:], xT_r[:, ksl, cs:cs + TCH])
                pieces.append(xc)
                if c == 0:
                    nc.sync.dma_start(w_q[:, ksl, :], wq_r[:, ksl, :])
                    nc.sync.dma_start(w_k[:, ksl, :], wk_r[:, ksl, :])
                    nc.sync.dma_start(w_v[:, ksl, :], wv_r[:, ksl, :])
            if c == 0:
                nc.sync.dma_start(ctab[:], ct_d[:])
                nc.sync.dma_start(stab[:], st_d[:])
                nc.sync.dma_start(roll[:], roll_d[:])
                nc.sync.dma_start(ones[:], ones_d[:])
                nc.sync.dma_start(tri[:], tri_d[:])
            return pieces

        def proj_chunk(c, pieces):
            """q/k/v projections + RoPE for t-chunk c."""
            cs = c * TCH
            qc = qpool.tile([128, H_LOC, TCH], f32r, tag="qc", name="qc")
            for w_sb, dst in ((w_q, qc), (w_k, kt)):
                for h in range(H_LOC):
                    dsl = dst[:, h, :] if dst is qc else dst[:, h, cs:cs + TCH]
                    # chunk 0 is DMA-paced: borrow the idle attention pools so
                    # all 8 projection groups can accumulate concurrently and
                    # every arriving piece has ready matmuls
                    ps = ps_tile(psum_ot if (c == 0 and dst is kt) else None)
                    for ko in range(KO):
                        nc.tensor.matmul(
                            ps,
                            lhsT=w_sb[:, ko, h * 128:(h + 1) * 128],
                            rhs=pieces[ko // XP][:, ko % XP, :],
                            start=(ko == 0),
                            stop=(ko == KO - 1),
                        )
                    nc.scalar.copy(out=dsl, in_=ps)

            # RoPE: y = x*C + roll64(x)*S' (only via PE roll + 3 DVE ops)
            for srct in (qc, kt):
                for h in range(H_LOC):
                    sl = srct[:, h, :] if srct is qc else srct[:, h, cs:cs + TCH]
                    rolled = ps_tile()
                    nc.tensor.matmul(rolled, lhsT=roll, rhs=sl,
                                     start=True, stop=True)
                    a = rtmp.tile([128, TCH], f32, tag="ra", name="ra")
                    b = rtmp.tile([128, TCH], f32, tag="rb", name="rb")
                    nc.vector.tensor_mul(out=a, in0=sl, in1=ctab[:, cs:cs + TCH])
                    nc.vector.tensor_mul(out=b, in0=rolled, in1=stab[:, cs:cs + TCH])
                    nc.vector.tensor_add(out=sl, in0=a, in1=b)
            vmix = mix_tile()
            for tt in range(TCH // 128):
                gt = c * (TCH // 128) + tt
                ps = vmix[:, tt // 2, (tt % 2) * HD_LOC:(tt % 2 + 1) * HD_LOC]
                for ko in range(KO):
                    nc.tensor.matmul(
                        ps[:, :HD_LOC],
                        lhsT=pieces[ko // XP][:, ko % XP, tt * 128:(tt + 1) * 128],
                        rhs=w_v[:, ko, :],
                        start=(ko == 0),
                        stop=(ko == KO - 1),
                    )
                nc.scalar.copy(out=vt[:, gt, :], in_=ps[:, :HD_LOC])

            return qc

        def attn_span(q0, W, qc, off, yc):
            """Causal attention for queries [q0, q0+W), heads interleaved.

            q0 must be 128-aligned; W in {256, 512}. qc holds the chunk's
            roped queries; off is q0's offset within qc/yc."""
            d0 = q0 // 128          # first diagonal j-tile
            n_jt = d0 + W // 128
            ots = [ps_tile(psum_ot) for _ in range(H_LOC)]
            vecsums = [[spool.tile([128, TCH], f32r, tag=f"vecsum{par}",
                                   name="vecsum")
                        for par in range(2)] for _ in range(H_LOC)]
            for jt in range(n_jt):
                pair = mix_tile()
                m = jt - d0
                # diagonal block: cols < 128m fully masked -- never written,
                # never read (partial-width ops)
                lo = 128 * m if m > 0 else 0
                # score matmul skips dead columns too, but only while the
                # moving dim stays >= 256 (full fp32r rate)
                slo = lo if W - lo >= 256 else 0
                for h in range(H_LOC):
                    nc.tensor.matmul(
                        pair[:, h, slo:W],
                        lhsT=kt[:, h, jt * 128:(jt + 1) * 128],
                        rhs=qc[:, h, off + slo:off + W],
                        start=True,
                        stop=True,
                    )
                pt = ptpool.tile([128, H_LOC, TCH], f32r, tag="pt", name="pt")
                # both heads in ONE activation call (strided AP when lo > 0)
                nc.scalar.activation(out=pt[:, :, lo:W], in_=pair[:, :, lo:W],
                                     func=EXP, scale=SCALE)
                for h in range(H_LOC):
                    if m >= 0:
                        nc.vector.tensor_mul(
                            out=pt[:, h, 128 * m:128 * (m + 1)],
                            in0=pt[:, h, 128 * m:128 * (m + 1)],
                            in1=tri[:],
                        )
                    # spans starting at q0=0: jt==1 is diagonal (cols < 128
                    # unwritten), so a full-width init copy would ingest
                    # garbage -- single DVE accumulator there. Other spans
                    # split across DVE (even jt) and GPSIMD (odd jt).
                    par = jt % 2 if d0 >= 2 else 0
                    vs = vecsums[h][par]
                    eng = nc.vector if par == 0 else nc.gpsimd
                    if jt < (2 if d0 >= 2 else 1):
                        eng.tensor_copy(out=vs[:, :W], in_=pt[:, h, :W])
                    else:
                        eng.tensor_add(out=vs[:, lo:W], in0=vs[:, lo:W],
                                       in1=pt[:, h, lo:W])
                    nc.tensor.matmul(
                        ots[h][:, lo:W],
                        lhsT=vt[:, jt, h * 128:(h + 1) * 128],
                        rhs=pt[:, h, lo:W],
                        start=(jt == 0),
                        stop=(jt == n_jt - 1),
                        skip_group_check=(lo > 0),
                    )
            for h in range(H_LOC):
                # denominator: all-ones matmul -> column sums on all partitions
                den = mix_tile()[:, 0, :W]
                if d0 >= 2:
                    nc.tensor.matmul(den, lhsT=ones, rhs=vecsums[h][0][:, :W],
                                     start=True, stop=False)
                    nc.tensor.matmul(den, lhsT=ones, rhs=vecsums[h][1][:, :W],
                                     start=False, stop=True)
                else:
                    nc.tensor.matmul(den, lhsT=ones, rhs=vecsums[h][0][:, :W],
                                     start=True, stop=True)
                recipb = rtmp.tile([128, TCH], f32, tag="recipb", name="recipb")
                nc.vector.reciprocal(out=recipb[:, :W], in_=den)
                nc.vector.tensor_mul(out=yc[:, h, off:off + W],
                                     in0=ots[h][:, :W], in1=recipb[:, :W])

        def cproj_span(q0, W, yc, off, copy_eng=None):
            """Partial c_proj (this core's hd columns) for rows [q0, q0+W)."""
            if q0 == 0:
                nc.sync.dma_start(w_o[:], wo_r)
            for tt in range(W // 128):
                gt = q0 // 128 + tt
                for nck in range(D // 512):
                    ps = mix_tile()[:, 0, :]
                    for h in range(H_LOC):
                        nc.tensor.matmul(
                            ps,
                            lhsT=yc[:, h, off + tt * 128:off + (tt + 1) * 128],
                            rhs=w_o[:, h, nck * 512:(nck + 1) * 512],
                            start=(h == 0),
                            stop=(h == H_LOC - 1),
                        )
                    ob = opool.tile([128, 512], f32, tag="ob", name="ob")
                    if copy_eng is None:
                        nc.scalar.copy(out=ob[:], in_=ps)
                    else:
                        copy_eng.tensor_copy(out=ob[:], in_=ps)
                    nc.sync.dma_start(
                        out_d[gt * 128:(gt + 1) * 128,
                              nck * 512:(nck + 1) * 512],
                        ob[:],
                    )

        # Emission order: projections stream in chunk order; each attention
        # chunk is emitted as soon as its projections exist, EXCEPT chunk 0
        # (the smallest, 4 j-tiles) which is saved for the tail so the
        # ACT-bound final attention stretch is as short as possible.
        pieces = issue_x(0)
        for c in range(N_CH):
            qc = proj_chunk(c, pieces)
            if c + 1 < N_CH:
                pieces = issue_x(c + 1)
            yc = ypool.tile([128, H_LOC, TCH], f32r, tag="yc", name="yc")
            attn_span(c * TCH, TCH, qc, 0, yc)
            cproj_span(c * TCH, TCH, yc, 0)

    nc.compile()
    _CACHE["nc"] = nc
    return nc


def host_inputs(x, Wq, Wk, Wv, Wo):
    """Per-core input dicts (host-side shard + transpose + table prep)."""
    x2 = np.ascontiguousarray(x.reshape(T, D).T).astype(np.float32)  # (D, T)

    half = DH // 2  # 64
    af = (1.0 / 1024.0) ** np.linspace(0.0, 1.0, DH // 4, dtype=np.float32)
    af = np.concatenate([af, np.zeros(DH // 4, np.float32)])         # (64,)
    theta = np.arange(T, dtype=np.float32)[:, None] * af[None, :]    # (T, 64)
    cos = np.cos(theta).T.astype(np.float32)                         # (64, T)
    sin = np.sin(theta).T.astype(np.float32)
    ctab = np.concatenate([cos, cos], axis=0)                        # (128, T)
    stab = np.concatenate([sin, -sin], axis=0)

    roll = np.zeros((128, 128), np.float32)
    for p in range(128):
        roll[p, (p + half) % 128] = 1.0
    ones = np.ones((128, 128), np.float32)
    tri = np.triu(np.ones((128, 128), np.float32))  # tri[j, i] = i >= j

    shared = {
        "xT": x2, "ctab": ctab, "stab": stab,
        "roll": roll, "ones": ones, "tri": tri,
    }
    in_maps = []
    for c in range(N_CORES):
        sl = slice(c * HD_LOC, (c + 1) * HD_LOC)
        in_maps.append({
            **shared,
            "wqT": np.ascontiguousarray(Wq[sl, :].T),
            "wkT": np.ascontiguousarray(Wk[sl, :].T),
            "wvT": np.ascontiguousarray(Wv[sl, :].T),
            "woT": np.ascontiguousarray((Wo[:, sl] / 3.0).T),
        })
    return in_maps


def _get_runner():
    """Build the program + a persistent jitted SPMD executable (once)."""
    if "runner" in _CACHE:
        return _CACHE["runner"]

    import jax
    import concourse.mybir as mybir
    from concourse.bass2jax import (
        _bass_exec_p,
        install_neuronx_cc_hook,
        partition_id_tensor,
    )
    from jax.experimental.shard_map import shard_map
    from jax.sharding import Mesh, PartitionSpec

    nc = build_program()
    install_neuronx_cc_hook()
    assert nc.dbg_addr is None
    pid_name = nc.partition_id_tensor.name if nc.partition_id_tensor else None

    in_names, out_names, out_avals, zero_outs = [], [], [], []
    for alloc in nc.m.functions[0].allocations:
        if not isinstance(alloc, mybir.MemoryLocationSet):
            continue
        name = alloc.memorylocations[0].name
        if alloc.kind == "ExternalInput":
            if name != pid_name:
                in_names.append(name)
        elif alloc.kind == "ExternalOutput":
            out_names.append(name)
            shape = tuple(alloc.tensor_shape)
            dtype = mybir.dt.np(alloc.dtype)
            out_avals.append(jax.core.ShapedArray(shape, dtype))
            zero_outs.append(np.zeros(shape, dtype))
    n_params = len(in_names)
    all_names = list(in_names) + list(out_names)
    if pid_name is not None:
        all_names.append(pid_name)
    donate = tuple(range(n_params, n_params + len(out_names)))

    def _body(*args):
        operands = list(args)
        if pid_name is not None:
            operands.append(partition_id_tensor())
        outs = _bass_exec_p.bind(
            *operands,
            out_avals=tuple(out_avals),
            in_names=tuple(all_names),
            out_names=tuple(out_names),
            lowering_input_output_aliases=(),
            sim_require_finite=True,
            sim_require_nnan=True,
            nc=nc,
        )
        return tuple(outs)

    devices = jax.devices()[:N_CORES]
    mesh = Mesh(np.asarray(devices), ("core",))
    in_specs = (PartitionSpec("core"),) * (n_params + len(out_names))
    out_specs = (PartitionSpec("core"),) * len(out_names)
    fn = jax.jit(
        shard_map(_body, mesh=mesh, in_specs=in_specs, out_specs=out_specs,
                  check_rep=False),
        donate_argnums=donate,
        keep_unused=True,
    )
    runner = (fn, in_names, out_names, out_avals, zero_outs)
    _CACHE["runner"] = runner
    return runner


def run_spmd(in_maps):
    """Execute the SPMD program; returns per-core output dicts."""
    fn, in_names, out_names, out_avals, zero_outs = _get_runner()
    concat_in = [
        np.concatenate([np.asarray(in_maps[c][n]) for c in range(N_CORES)], axis=0)
        for n in in_names
    ]
    concat_zeros = [
        np.zeros((N_CORES * z.shape[0], *z.shape[1:]), z.dtype) for z in zero_outs
    ]
    out_arrs = fn(*concat_in, *concat_zeros)
    return [
        {n: np.asarray(out_arrs[i]).reshape(N_CORES, *out_avals[i].shape)[c]
         for i, n in enumerate(out_names)}
        for c in range(N_CORES)
    ]


def kernel(x, Wq, Wk, Wv, Wo):
    in_maps = host_inputs(np.asarray(x), np.asarray(Wq), np.asarray(Wk),
                          np.asarray(Wv), np.asarray(Wo))
    results = run_spmd(in_maps)
    out = results[0]["outp"].astype(np.float64)
    for c in range(1, N_CORES):
        out += results[c]["outp"]
    return out.astype(np.float32).reshape(1, T, D)


# revision 48
# speedup vs baseline: 172557.8572x; 1.0066x over previous
"""Causal self-attention with RoPE, tensor-parallel over heads on 8 TRN2 NeuronCores.

Model (from the reference):
    q/k/v = x @ W{q,k,v}.T          x: (1, 2048, 2048), 16 heads x 128 head_dim
    rope(q), rope(k)                half-rotation, 32 nonzero freqs
    causal softmax(q k^T / sqrt(128)) @ v
    out = (y / 3) @ Wo.T

Sharding: 2 heads per core. Each core computes its heads' q/k/v projections,
attention, and a partial c_proj (its 256 columns of the hd contraction);
the host sums the 8 partial outputs (the "all-reduce after c_proj").

Per-core kernel layout choices:
  - Everything transposed so the contraction dim is always on partitions:
    host supplies xT (D, T) plus pre-transposed weight slices.
  - Scores computed transposed (S^T[j, i] blocks) so the P @ V matmul needs
    no transposes: OT[d, i] = sum_j V[j, d]^T P^T[j, i] is produced directly
    in the layout c_proj wants.
  - Softmax without max-subtraction (scores are provably tiny: |s| < ~2),
    denominator via DVE accumulation + one all-ones matmul (broadcast sum).
  - RoPE in transposed layout via a 64-partition roll matmul + 3 DVE ops.
  - All matmuls in float32r (full PE rate at moving dim >= 256).
"""

import numpy as np

T = 2048
D = 2048
H = 16
DH = 128
N_CORES = 8
H_LOC = H // N_CORES          # heads per core = 2
HD_LOC = H_LOC * DH           # local head dims = 256
TCH = 512                     # query-chunk width
N_CH = T // TCH               # 4 chunks
KO = D // 128                 # 16 contraction subtiles
XP = 2                        # xT streamed in pieces of 2 k-subtiles
SCALE = (DH ** 0.5) / DH      # 1/sqrt(128)

_CACHE = {}


def build_program():
    """Build (once) the single-core Bass program shared by all 8 cores."""
    if "nc" in _CACHE:
        return _CACHE["nc"]

    from contextlib import ExitStack

    import concourse.bacc as bacc
    import concourse.mybir as mybir
    import concourse.tile as tile

    f32 = mybir.dt.float32
    f32r = mybir.dt.float32r
    bf16 = mybir.dt.bfloat16
    EXP = mybir.ActivationFunctionType.Exp

    nc = bacc.Bacc("TRN2", target_bir_lowering=False)

    xT_d = nc.dram_tensor("xT", (D, T), f32r, kind="ExternalInput")
    wq_d = nc.dram_tensor("wqT", (D, HD_LOC), f32r, kind="ExternalInput")
    wk_d = nc.dram_tensor("wkT", (D, HD_LOC), f32r, kind="ExternalInput")
    wv_d = nc.dram_tensor("wvT", (D, HD_LOC), f32r, kind="ExternalInput")
    wo_d = nc.dram_tensor("woT", (HD_LOC, D), f32r, kind="ExternalInput")
    ct_d = nc.dram_tensor("ctab", (128, T), f32, kind="ExternalInput")
    st_d = nc.dram_tensor("stab", (128, T), f32, kind="ExternalInput")
    roll_d = nc.dram_tensor("roll", (128, 128), f32r, kind="ExternalInput")
    ones_d = nc.dram_tensor("ones", (128, 128), f32r, kind="ExternalInput")
    tri_d = nc.dram_tensor("tri", (128, 128), f32r, kind="ExternalInput")
    out_d = nc.dram_tensor("outp", (T, D), f32, kind="ExternalOutput")

    xT_r = xT_d[:].rearrange("(ko p) t -> p ko t", p=128)
    wq_r = wq_d[:].rearrange("(ko p) m -> p ko m", p=128)
    wk_r = wk_d[:].rearrange("(ko p) m -> p ko m", p=128)
    wv_r = wv_d[:].rearrange("(ko p) m -> p ko m", p=128)
    wo_r = wo_d[:].rearrange("(h p) d -> p h d", p=128)

    with tile.TileContext(nc) as tc, ExitStack() as ctx:
        persist = ctx.enter_context(tc.tile_pool(name="persist", bufs=1))
        qpool = ctx.enter_context(tc.tile_pool(name="qpool", bufs=2))
        ypool = ctx.enter_context(tc.tile_pool(name="ypool", bufs=2))
        xpool = ctx.enter_context(tc.tile_pool(name="xpool", bufs=10))
        ptpool = ctx.enter_context(tc.tile_pool(name="ptpool", bufs=3))
        rtmp = ctx.enter_context(tc.tile_pool(name="rtmp", bufs=1))
        spool = ctx.enter_context(tc.tile_pool(name="spool", bufs=2))
        opool = ctx.enter_context(tc.tile_pool(name="opool", bufs=6))
        psum_p = ctx.enter_context(tc.tile_pool(name="psum_p", bufs=2, space="PSUM"))
        psum_mix = ctx.enter_context(tc.tile_pool(name="psum_mix", bufs=2, space="PSUM"))
        psum_ot = ctx.enter_context(tc.tile_pool(name="psum_ot", bufs=2, space="PSUM"))

        def ps_tile(pool=None):
            return (pool or psum_p).tile([128, TCH], f32, tag="ps", name="ps")

        def mix_tile():
            return psum_mix.tile([128, H_LOC, TCH], f32, tag="mix", name="mix")

        # --- resident tensors ---
        w_q = persist.tile([128, KO, HD_LOC], f32r, tag="w_q")
        w_k = persist.tile([128, KO, HD_LOC], f32r, tag="w_k")
        w_v = persist.tile([128, KO, HD_LOC], f32r, tag="w_v")
        w_o = persist.tile([128, H_LOC, D], f32r, tag="w_o")
        kt = persist.tile([128, H_LOC, T], f32r, tag="kt")
        vt = persist.tile([128, KO, HD_LOC], f32r, tag="vt")
        ctab = persist.tile([128, T], f32, tag="ctab")
        stab = persist.tile([128, T], f32, tag="stab")
        roll = persist.tile([128, 128], f32r, tag="roll")
        ones = persist.tile([128, 128], f32r, tag="ones")
        tri = persist.tile([128, 128], f32r, tag="tri")

        def issue_x(c):
            """Queue the xT piece DMAs for chunk c (weights too on chunk 0)."""
            cs = c * TCH
            pieces = []
            for kp in range(KO // XP):
                ksl = slice(kp * XP, (kp + 1) * XP)
                xc = xpool.tile([128, XP, TCH], f32r, tag="xc", name="xc")
                nc.sync.dma_start(xc[:], xT_r[:, ksl, cs:cs + TCH])
                pieces.append(xc)
                if c == 0:
                    nc.sync.dma_start(w_q[:, ksl, :], wq_r[:, ksl, :])
                    nc.sync.dma_start(w_k[:, ksl, :], wk_r[:, ksl, :])
                    nc.sync.dma_start(w_v[:, ksl, :], wv_r[:, ksl, :])
            if c == 0:
                nc.sync.dma_start(ctab[:], ct_d[:])
                nc.sync.dma_start(stab[:], st_d[:])
                nc.sync.dma_start(roll[:], roll_d[:])
                nc.sync.dma_start(ones[:], ones_d[:])
                nc.sync.dma_start(tri[:], tri_d[:])
            return pieces

        def proj_chunk(c, pieces):
            """q/k/v projections + RoPE for t-chunk c."""
            cs = c * TCH
            qc = qpool.tile([128, H_LOC, TCH], f32r, tag="qc", name="qc")
            for w_sb, dst in ((w_q, qc), (w_k, kt)):
                for h in range(H_LOC):
                    dsl = dst[:, h, :] if dst is qc else dst[:, h, cs:cs + TCH]
                    # k-groups accumulate in the attention ot pool (idle during
                    # projections) so q/k/roll don't serialize through psum_p
                    ps = ps_tile(psum_ot if dst is kt else None)
                    for ko in range(KO):
                        nc.tensor.matmul(
                            ps,
                            lhsT=w_sb[:, ko, h * 128:(h + 1) * 128],
                            rhs=pieces[ko // XP][:, ko % XP, :],
                            start=(ko == 0),
                            stop=(ko == KO - 1),
                        )
                    nc.scalar.copy(out=dsl, in_=ps)

            # RoPE: y = x*C + roll64(x)*S' (only via PE roll + 3 DVE ops)
            for srct in (qc, kt):
                for h in range(H_LOC):
                    sl = srct[:, h, :] if srct is qc else srct[:, h, cs:cs + TCH]
                    rolled = ps_tile()
                    nc.tensor.matmul(rolled, lhsT=roll, rhs=sl,
                                     start=True, stop=True)
                    a = rtmp.tile([128, TCH], f32, tag="ra", name="ra")
                    b = rtmp.tile([128, TCH], f32, tag="rb", name="rb")
                    nc.vector.tensor_mul(out=a, in0=sl, in1=ctab[:, cs:cs + TCH])
                    nc.vector.tensor_mul(out=b, in0=rolled, in1=stab[:, cs:cs + TCH])
                    nc.vector.tensor_add(out=sl, in0=a, in1=b)
            vmix = mix_tile()
            for tt in range(TCH // 128):
                gt = c * (TCH // 128) + tt
                ps = vmix[:, tt // 2, (tt % 2) * HD_LOC:(tt % 2 + 1) * HD_LOC]
                for ko in range(KO):
                    nc.tensor.matmul(
                        ps[:, :HD_LOC],
                        lhsT=pieces[ko // XP][:, ko % XP, tt * 128:(tt + 1) * 128],
                        rhs=w_v[:, ko, :],
                        start=(ko == 0),
                        stop=(ko == KO - 1),
                    )
                nc.scalar.copy(out=vt[:, gt, :], in_=ps[:, :HD_LOC])

            return qc

        def attn_span(q0, W, qc, off, yc):
            """Causal attention for queries [q0, q0+W), heads interleaved.

            q0 must be 128-aligned; W in {256, 512}. qc holds the chunk's
            roped queries; off is q0's offset within qc/yc."""
            d0 = q0 // 128          # first diagonal j-tile
            n_jt = d0 + W // 128
            ots = [ps_tile(psum_ot) for _ in range(H_LOC)]
            vecsums = [[spool.tile([128, TCH], f32r, tag=f"vecsum{par}",
                                   name="vecsum")
                        for par in range(2)] for _ in range(H_LOC)]
            for jt in range(n_jt):
                pair = mix_tile()
                m = jt - d0
                # diagonal block: cols < 128m fully masked -- never written,
                # never read (partial-width ops)
                lo = 128 * m if m > 0 else 0
                # score matmul skips dead columns too, but only while the
                # moving dim stays >= 256 (full fp32r rate)
                slo = lo if W - lo >= 256 else 0
                for h in range(H_LOC):
                    nc.tensor.matmul(
                        pair[:, h, slo:W],
                        lhsT=kt[:, h, jt * 128:(jt + 1) * 128],
                        rhs=qc[:, h, off + slo:off + W],
                        start=True,
                        stop=True,
                    )
                pt = ptpool.tile([128, H_LOC, TCH], f32r, tag="pt", name="pt")
                # both heads in ONE activation call (strided AP when lo > 0)
                nc.scalar.activation(out=pt[:, :, lo:W], in_=pair[:, :, lo:W],
                                     func=EXP, scale=SCALE)
                for h in range(H_LOC):
                    if m >= 0:
                        nc.vector.tensor_mul(
                            out=pt[:, h, 128 * m:128 * (m + 1)],
                            in0=pt[:, h, 128 * m:128 * (m + 1)],
                            in1=tri[:],
                        )
                    # spans starting at q0=0: jt==1 is diagonal (cols < 128
                    # unwritten), so a full-width init copy would ingest
                    # garbage -- single DVE accumulator there. Other spans
                    # split across DVE (even jt) and GPSIMD (odd jt).
                    par = jt % 2 if d0 >= 2 else 0
                    vs = vecsums[h][par]
                    eng = nc.vector if par == 0 else nc.gpsimd
                    if jt < (2 if d0 >= 2 else 1):
                        eng.tensor_copy(out=vs[:, :W], in_=pt[:, h, :W])
                    else:
                        eng.tensor_add(out=vs[:, lo:W], in0=vs[:, lo:W],
                                       in1=pt[:, h, lo:W])
                    nc.tensor.matmul(
                        ots[h][:, lo:W],
                        lhsT=vt[:, jt, h * 128:(h + 1) * 128],
                        rhs=pt[:, h, lo:W],
                        start=(jt == 0),
                        stop=(jt == n_jt - 1),
                        skip_group_check=(lo > 0),
                    )
            for h in range(H_LOC):
                # denominator: all-ones matmul -> column sums on all partitions
                den = mix_tile()[:, 0, :W]
                if d0 >= 2:
                    nc.tensor.matmul(den, lhsT=ones, rhs=vecsums[h][0][:, :W],
                                     start=True, stop=False)
                    nc.tensor.matmul(den, lhsT=ones, rhs=vecsums[h][1][:, :W],
                                     start=False, stop=True)
                else:
                    nc.tensor.matmul(den, lhsT=ones, rhs=vecsums[h][0][:, :W],
                                     start=True, stop=True)
                recipb = rtmp.tile([128, TCH], f32, tag="recipb", name="recipb")
                nc.vector.reciprocal(out=recipb[:, :W], in_=den)
                nc.vector.tensor_mul(out=yc[:, h, off:off + W],
                                     in0=ots[h][:, :W], in1=recipb[:, :W])

        def cproj_span(q0, W, yc, off, copy_eng=None):
            """Partial c_proj (this core's hd columns) for rows [q0, q0+W)."""
            if q0 == 0:
                nc.sync.dma_start(w_o[:], wo_r)
            for tt in range(W // 128):
                gt = q0 // 128 + tt
                for nck in range(D // 512):
                    ps = mix_tile()[:, 0, :]
                    for h in range(H_LOC):
                        nc.tensor.matmul(
                            ps,
                            lhsT=yc[:, h, off + tt * 128:off + (tt + 1) * 128],
                            rhs=w_o[:, h, nck * 512:(nck + 1) * 512],
                            start=(h == 0),
                            stop=(h == H_LOC - 1),
                        )
                    ob = opool.tile([128, 512], f32, tag="ob", name="ob")
                    if copy_eng is None:
                        nc.scalar.copy(out=ob[:], in_=ps)
                    else:
                        copy_eng.tensor_copy(out=ob[:], in_=ps)
                    nc.sync.dma_start(
                        out_d[gt * 128:(gt + 1) * 128,
                              nck * 512:(nck + 1) * 512],
                        ob[:],
                    )

        # Emission order: projections stream in chunk order; each attention
        # chunk is emitted as soon as its projections exist, EXCEPT chunk 0
        # (the smallest, 4 j-tiles) which is saved for the tail so the
        # ACT-bound final attention stretch is as short as possible.
        pieces = issue_x(0)
        for c in range(N_CH):
            qc = proj_chunk(c, pieces)
            if c + 1 < N_CH:
                pieces = issue_x(c + 1)
            yc = ypool.tile([128, H_LOC, TCH], f32r, tag="yc", name="yc")
            attn_span(c * TCH, TCH, qc, 0, yc)
            cproj_span(c * TCH, TCH, yc, 0)

    nc.compile()
    _CACHE["nc"] = nc
    return nc


def host_inputs(x, Wq, Wk, Wv, Wo):
    """Per-core input dicts (host-side shard + transpose + table prep)."""
    x2 = np.ascontiguousarray(x.reshape(T, D).T).astype(np.float32)  # (D, T)

    half = DH // 2  # 64
    af = (1.0 / 1024.0) ** np.linspace(0.0, 1.0, DH // 4, dtype=np.float32)
    af = np.concatenate([af, np.zeros(DH // 4, np.float32)])         # (64,)
    theta = np.arange(T, dtype=np.float32)[:, None] * af[None, :]    # (T, 64)
    cos = np.cos(theta).T.astype(np.float32)                         # (64, T)
    sin = np.sin(theta).T.astype(np.float32)
    ctab = np.concatenate([cos, cos], axis=0)                        # (128, T)
    stab = np.concatenate([sin, -sin], axis=0)

    roll = np.zeros((128, 128), np.float32)
    for p in range(128):
        roll[p, (p + half) % 128] = 1.0
    ones = np.ones((128, 128), np.float32)
    tri = np.triu(np.ones((128, 128), np.float32))  # tri[j, i] = i >= j

    shared = {
        "xT": x2, "ctab": ctab, "stab": stab,
        "roll": roll, "ones": ones, "tri": tri,
    }
    in_maps = []
    for c in range(N_CORES):
        sl = slice(c * HD_LOC, (c + 1) * HD_LOC)
        in_maps.append({
            **shared,
            "wqT": np.ascontiguousarray(Wq[sl, :].T),
            "wkT": np.ascontiguousarray(Wk[sl, :].T),
            "wvT": np.ascontiguousarray(Wv[sl, :].T),
            "woT": np.ascontiguousarray((Wo[:, sl] / 3.0).T),
        })
    return in_maps


def _get_runner():
    """Build the program + a persistent jitted SPMD executable (once)."""
    if "runner" in _CACHE:
        return _CACHE["runner"]

    import jax
    import concourse.mybir as mybir
    from concourse.bass2jax import (
        _bass_exec_p,
        install_neuronx_cc_hook,
        partition_id_tensor,
    )
    from jax.experimental.shard_map import shard_map
    from jax.sharding import Mesh, PartitionSpec

    nc = build_program()
    install_neuronx_cc_hook()
    assert nc.dbg_addr is None
    pid_name = nc.partition_id_tensor.name if nc.partition_id_tensor else None

    in_names, out_names, out_avals, zero_outs = [], [], [], []
    for alloc in nc.m.functions[0].allocations:
        if not isinstance(alloc, mybir.MemoryLocationSet):
            continue
        name = alloc.memorylocations[0].name
        if alloc.kind == "ExternalInput":
            if name != pid_name:
                in_names.append(name)
        elif alloc.kind == "ExternalOutput":
            out_names.append(name)
            shape = tuple(alloc.tensor_shape)
            dtype = mybir.dt.np(alloc.dtype)
            out_avals.append(jax.core.ShapedArray(shape, dtype))
            zero_outs.append(np.zeros(shape, dtype))
    n_params = len(in_names)
    all_names = list(in_names) + list(out_names)
    if pid_name is not None:
        all_names.append(pid_name)
    donate = tuple(range(n_params, n_params + len(out_names)))

    def _body(*args):
        operands = list(args)
        if pid_name is not None:
            operands.append(partition_id_tensor())
        outs = _bass_exec_p.bind(
            *operands,
            out_avals=tuple(out_avals),
            in_names=tuple(all_names),
            out_names=tuple(out_names),
            lowering_input_output_aliases=(),
            sim_require_finite=True,
            sim_require_nnan=True,
            nc=nc,
        )
        return tuple(outs)

    devices = jax.devices()[:N_CORES]
    mesh = Mesh(np.asarray(devices), ("core",))
    in_specs = (PartitionSpec("core"),) * (n_params + len(out_names))
    out_specs = (PartitionSpec("core"),) * len(out_names)
    fn = jax.jit(
        shard_map(_body, mesh=mesh, in_specs=in_specs, out_specs=out_specs,
                  check_rep=False),
        donate_argnums=donate,
        keep_unused=True,
    )
    runner = (fn, in_names, out_names, out_avals, zero_outs)
    _CACHE["runner"] = runner
    return runner


def run_spmd(in_maps):
    """Execute the SPMD program; returns per-core output dicts."""
    fn, in_names, out_names, out_avals, zero_outs = _get_runner()
    concat_in = [
        np.concatenate([np.asarray(in_maps[c][n]) for c in range(N_CORES)], axis=0)
        for n in in_names
    ]
    concat_zeros = [
        np.zeros((N_CORES * z.shape[0], *z.shape[1:]), z.dtype) for z in zero_outs
    ]
    out_arrs = fn(*concat_in, *concat_zeros)
    return [
        {n: np.asarray(out_arrs[i]).reshape(N_CORES, *out_avals[i].shape)[c]
         for i, n in enumerate(out_names)}
        for c in range(N_CORES)
    ]


def kernel(x, Wq, Wk, Wv, Wo):
    in_maps = host_inputs(np.asarray(x), np.asarray(Wq), np.asarray(Wk),
                          np.asarray(Wv), np.asarray(Wo))
    results = run_spmd(in_maps)
    out = results[0]["outp"].astype(np.float64)
    for c in range(1, N_CORES):
        out += results[c]["outp"]
    return out.astype(np.float32).reshape(1, T, D)


# revision 49
# speedup vs baseline: 175322.5612x; 1.0160x over previous
"""Causal self-attention with RoPE, tensor-parallel over heads on 8 TRN2 NeuronCores.

Model (from the reference):
    q/k/v = x @ W{q,k,v}.T          x: (1, 2048, 2048), 16 heads x 128 head_dim
    rope(q), rope(k)                half-rotation, 32 nonzero freqs
    causal softmax(q k^T / sqrt(128)) @ v
    out = (y / 3) @ Wo.T

Sharding: 2 heads per core. Each core computes its heads' q/k/v projections,
attention, and a partial c_proj (its 256 columns of the hd contraction);
the host sums the 8 partial outputs (the "all-reduce after c_proj").

Per-core kernel layout choices:
  - Everything transposed so the contraction dim is always on partitions:
    host supplies xT (D, T) plus pre-transposed weight slices.
  - Scores computed transposed (S^T[j, i] blocks) so the P @ V matmul needs
    no transposes: OT[d, i] = sum_j V[j, d]^T P^T[j, i] is produced directly
    in the layout c_proj wants.
  - Softmax without max-subtraction (scores are provably tiny: |s| < ~2),
    denominator via DVE accumulation + one all-ones matmul (broadcast sum).
  - RoPE in transposed layout via a 64-partition roll matmul + 3 DVE ops.
  - All matmuls in float32r (full PE rate at moving dim >= 256).
"""

import numpy as np

T = 2048
D = 2048
H = 16
DH = 128
N_CORES = 8
H_LOC = H // N_CORES          # heads per core = 2
HD_LOC = H_LOC * DH           # local head dims = 256
TCH = 512                     # query-chunk width
N_CH = T // TCH               # 4 chunks
KO = D // 128                 # 16 contraction subtiles
XP = 2                        # xT streamed in pieces of 2 k-subtiles
SCALE = (DH ** 0.5) / DH      # 1/sqrt(128)

_CACHE = {}


def build_program():
    """Build (once) the single-core Bass program shared by all 8 cores."""
    if "nc" in _CACHE:
        return _CACHE["nc"]

    from contextlib import ExitStack

    import concourse.bacc as bacc
    import concourse.mybir as mybir
    import concourse.tile as tile

    f32 = mybir.dt.float32
    f32r = mybir.dt.float32r
    bf16 = mybir.dt.bfloat16
    EXP = mybir.ActivationFunctionType.Exp

    nc = bacc.Bacc("TRN2", target_bir_lowering=False)

    xT_d = nc.dram_tensor("xT", (D, T), f32r, kind="ExternalInput")
    wq_d = nc.dram_tensor("wqT", (D, HD_LOC), f32r, kind="ExternalInput")
    wk_d = nc.dram_tensor("wkT", (D, HD_LOC), f32r, kind="ExternalInput")
    wv_d = nc.dram_tensor("wvT", (D, HD_LOC), f32r, kind="ExternalInput")
    wo_d = nc.dram_tensor("woT", (HD_LOC, D), f32r, kind="ExternalInput")
    ct_d = nc.dram_tensor("ctab", (128, T), f32, kind="ExternalInput")
    st_d = nc.dram_tensor("stab", (128, T), f32, kind="ExternalInput")
    roll_d = nc.dram_tensor("roll", (128, 128), f32r, kind="ExternalInput")
    ones_d = nc.dram_tensor("ones", (128, 128), f32r, kind="ExternalInput")
    tri_d = nc.dram_tensor("tri", (128, 128), f32r, kind="ExternalInput")
    out_d = nc.dram_tensor("outp", (T, D), f32, kind="ExternalOutput")

    xT_r = xT_d[:].rearrange("(ko p) t -> p ko t", p=128)
    wq_r = wq_d[:].rearrange("(ko p) m -> p ko m", p=128)
    wk_r = wk_d[:].rearrange("(ko p) m -> p ko m", p=128)
    wv_r = wv_d[:].rearrange("(ko p) m -> p ko m", p=128)
    wo_r = wo_d[:].rearrange("(h p) d -> p h d", p=128)

    with tile.TileContext(nc) as tc, ExitStack() as ctx:
        persist = ctx.enter_context(tc.tile_pool(name="persist", bufs=1))
        qpool = ctx.enter_context(tc.tile_pool(name="qpool", bufs=2))
        ypool = ctx.enter_context(tc.tile_pool(name="ypool", bufs=2))
        xpool = ctx.enter_context(tc.tile_pool(name="xpool", bufs=10))
        ptpool = ctx.enter_context(tc.tile_pool(name="ptpool", bufs=3))
        rtmp = ctx.enter_context(tc.tile_pool(name="rtmp", bufs=1))
        spool = ctx.enter_context(tc.tile_pool(name="spool", bufs=2))
        opool = ctx.enter_context(tc.tile_pool(name="opool", bufs=6))
        psum_p = ctx.enter_context(tc.tile_pool(name="psum_p", bufs=2, space="PSUM"))
        psum_mix = ctx.enter_context(tc.tile_pool(name="psum_mix", bufs=2, space="PSUM"))
        psum_ot = ctx.enter_context(tc.tile_pool(name="psum_ot", bufs=2, space="PSUM"))

        def ps_tile(pool=None):
            return (pool or psum_p).tile([128, TCH], f32, tag="ps", name="ps")

        def mix_tile():
            return psum_mix.tile([128, H_LOC, TCH], f32, tag="mix", name="mix")

        # --- resident tensors ---
        w_q = persist.tile([128, KO, HD_LOC], f32r, tag="w_q")
        w_k = persist.tile([128, KO, HD_LOC], f32r, tag="w_k")
        w_v = persist.tile([128, KO, HD_LOC], f32r, tag="w_v")
        w_o = persist.tile([128, H_LOC, D], f32r, tag="w_o")
        kt = persist.tile([128, H_LOC, T], f32r, tag="kt")
        vt = persist.tile([128, KO, HD_LOC], f32r, tag="vt")
        ctab = persist.tile([128, T], f32, tag="ctab")
        stab = persist.tile([128, T], f32, tag="stab")
        roll = persist.tile([128, 128], f32r, tag="roll")
        ones = persist.tile([128, 128], f32r, tag="ones")
        tri = persist.tile([128, 128], f32r, tag="tri")

        def issue_x(c):
            """Queue the xT piece DMAs for chunk c (weights too on chunk 0)."""
            cs = c * TCH
            pieces = []
            for kp in range(KO // XP):
                ksl = slice(kp * XP, (kp + 1) * XP)
                xc = xpool.tile([128, XP, TCH], f32r, tag="xc", name="xc")
                nc.sync.dma_start(xc[:], xT_r[:, ksl, cs:cs + TCH])
                pieces.append(xc)
                if c == 0:
                    nc.sync.dma_start(w_q[:, ksl, :], wq_r[:, ksl, :])
                    nc.sync.dma_start(w_k[:, ksl, :], wk_r[:, ksl, :])
                    nc.sync.dma_start(w_v[:, ksl, :], wv_r[:, ksl, :])
            if c == 0:
                nc.sync.dma_start(ctab[:], ct_d[:])
                nc.sync.dma_start(stab[:], st_d[:])
                nc.sync.dma_start(roll[:], roll_d[:])
                nc.sync.dma_start(ones[:], ones_d[:])
                nc.sync.dma_start(tri[:], tri_d[:])
            return pieces

        def proj_chunk(c, pieces, only=None, qc=None):
            """q/k/v projections + RoPE for t-chunk c.

            only="q": just the q projection + its rope (enables starting the
            chunk's early attention j-tiles before k/v exist).
            only="kv": the rest. None: everything."""
            cs = c * TCH
            if only != "kv":
                qc = qpool.tile([128, H_LOC, TCH], f32r, tag="qc", name="qc")
            wd = {"q": ((w_q, qc),), "kv": ((w_k, kt),)}.get(only,
                                                            ((w_q, qc), (w_k, kt)))
            for w_sb, dst in wd:
                for h in range(H_LOC):
                    dsl = dst[:, h, :] if dst is qc else dst[:, h, cs:cs + TCH]
                    # k-groups accumulate in the attention ot pool (idle during
                    # projections) so q/k/roll don't serialize through psum_p;
                    # in split mode that pool is live -- fall back to psum_p
                    ps = ps_tile(psum_ot if (dst is kt and only is None) else None)
                    for ko in range(KO):
                        nc.tensor.matmul(
                            ps,
                            lhsT=w_sb[:, ko, h * 128:(h + 1) * 128],
                            rhs=pieces[ko // XP][:, ko % XP, :],
                            start=(ko == 0),
                            stop=(ko == KO - 1),
                        )
                    nc.scalar.copy(out=dsl, in_=ps)

            if only == "q":
                rope_srcs, do_v = (qc,), False
            elif only == "kv":
                rope_srcs, do_v = (kt,), True
            else:
                rope_srcs, do_v = (qc, kt), True
            # RoPE: y = x*C + roll64(x)*S' (only via PE roll + 3 DVE ops)
            for srct in rope_srcs:
                for h in range(H_LOC):
                    sl = srct[:, h, :] if srct is qc else srct[:, h, cs:cs + TCH]
                    rolled = ps_tile()
                    nc.tensor.matmul(rolled, lhsT=roll, rhs=sl,
                                     start=True, stop=True)
                    a = rtmp.tile([128, TCH], f32, tag="ra", name="ra")
                    b = rtmp.tile([128, TCH], f32, tag="rb", name="rb")
                    nc.vector.tensor_mul(out=a, in0=sl, in1=ctab[:, cs:cs + TCH])
                    nc.vector.tensor_mul(out=b, in0=rolled, in1=stab[:, cs:cs + TCH])
                    nc.vector.tensor_add(out=sl, in0=a, in1=b)
            if not do_v:
                return qc
            # split mode overlaps attention (which owns mix/ot): v uses psum_p
            vmix = mix_tile() if only is None else None
            for tt in range(TCH // 128):
                gt = c * (TCH // 128) + tt
                if vmix is not None:
                    ps = vmix[:, tt // 2,
                              (tt % 2) * HD_LOC:(tt % 2 + 1) * HD_LOC]
                else:
                    ps = ps_tile()
                for ko in range(KO):
                    nc.tensor.matmul(
                        ps[:, :HD_LOC],
                        lhsT=pieces[ko // XP][:, ko % XP, tt * 128:(tt + 1) * 128],
                        rhs=w_v[:, ko, :],
                        start=(ko == 0),
                        stop=(ko == KO - 1),
                    )
                nc.scalar.copy(out=vt[:, gt, :], in_=ps[:, :HD_LOC])

            return qc

        def attn_span(q0, W, qc, off, yc, jt_lo=0, jt_hi=None,
                      state=None):
            """Causal attention for queries [q0, q0+W), heads interleaved.

            q0 must be 128-aligned; W in {256, 512}. qc holds the chunk's
            roped queries; off is q0's offset within qc/yc."""
            d0 = q0 // 128          # first diagonal j-tile
            n_jt = d0 + W // 128
            if state is None:
                ots = [ps_tile(psum_ot) for _ in range(H_LOC)]
                vecsums = [[spool.tile([128, TCH], f32r, tag=f"vecsum{par}",
                                       name="vecsum")
                            for par in range(2)] for _ in range(H_LOC)]
            else:
                ots, vecsums = state
            if jt_hi is None:
                jt_hi = n_jt
            for jt in range(jt_lo, jt_hi):
                pair = mix_tile()
                m = jt - d0
                # diagonal block: cols < 128m fully masked -- never written,
                # never read (partial-width ops)
                lo = 128 * m if m > 0 else 0
                # score matmul skips dead columns too, but only while the
                # moving dim stays >= 256 (full fp32r rate)
                slo = lo if W - lo >= 256 else 0
                for h in range(H_LOC):
                    nc.tensor.matmul(
                        pair[:, h, slo:W],
                        lhsT=kt[:, h, jt * 128:(jt + 1) * 128],
                        rhs=qc[:, h, off + slo:off + W],
                        start=True,
                        stop=True,
                    )
                pt = ptpool.tile([128, H_LOC, TCH], f32r, tag="pt", name="pt")
                # both heads in ONE activation call (strided AP when lo > 0)
                nc.scalar.activation(out=pt[:, :, lo:W], in_=pair[:, :, lo:W],
                                     func=EXP, scale=SCALE)
                for h in range(H_LOC):
                    if m >= 0:
                        nc.vector.tensor_mul(
                            out=pt[:, h, 128 * m:128 * (m + 1)],
                            in0=pt[:, h, 128 * m:128 * (m + 1)],
                            in1=tri[:],
                        )
                    # spans starting at q0=0: jt==1 is diagonal (cols < 128
                    # unwritten), so a full-width init copy would ingest
                    # garbage -- single DVE accumulator there. Other spans
                    # split across DVE (even jt) and GPSIMD (odd jt).
                    par = jt % 2 if d0 >= 2 else 0
                    vs = vecsums[h][par]
                    eng = nc.vector if par == 0 else nc.gpsimd
                    if jt < (2 if d0 >= 2 else 1):
                        eng.tensor_copy(out=vs[:, :W], in_=pt[:, h, :W])
                    else:
                        eng.tensor_add(out=vs[:, lo:W], in0=vs[:, lo:W],
                                       in1=pt[:, h, lo:W])
                    nc.tensor.matmul(
                        ots[h][:, lo:W],
                        lhsT=vt[:, jt, h * 128:(h + 1) * 128],
                        rhs=pt[:, h, lo:W],
                        start=(jt == 0),
                        stop=(jt == n_jt - 1),
                        skip_group_check=(lo > 0),
                    )
            if jt_hi < n_jt:
                return (ots, vecsums)
            for h in range(H_LOC):
                # denominator: all-ones matmul -> column sums on all partitions
                den = mix_tile()[:, 0, :W]
                if d0 >= 2:
                    nc.tensor.matmul(den, lhsT=ones, rhs=vecsums[h][0][:, :W],
                                     start=True, stop=False)
                    nc.tensor.matmul(den, lhsT=ones, rhs=vecsums[h][1][:, :W],
                                     start=False, stop=True)
                else:
                    nc.tensor.matmul(den, lhsT=ones, rhs=vecsums[h][0][:, :W],
                                     start=True, stop=True)
                recipb = rtmp.tile([128, TCH], f32, tag="recipb", name="recipb")
                nc.vector.reciprocal(out=recipb[:, :W], in_=den)
                nc.vector.tensor_mul(out=yc[:, h, off:off + W],
                                     in0=ots[h][:, :W], in1=recipb[:, :W])

        def cproj_span(q0, W, yc, off, copy_eng=None):
            """Partial c_proj (this core's hd columns) for rows [q0, q0+W)."""
            if q0 == 0:
                nc.sync.dma_start(w_o[:], wo_r)
            for tt in range(W // 128):
                gt = q0 // 128 + tt
                for nck in range(D // 512):
                    ps = mix_tile()[:, 0, :]
                    for h in range(H_LOC):
                        nc.tensor.matmul(
                            ps,
                            lhsT=yc[:, h, off + tt * 128:off + (tt + 1) * 128],
                            rhs=w_o[:, h, nck * 512:(nck + 1) * 512],
                            start=(h == 0),
                            stop=(h == H_LOC - 1),
                        )
                    ob = opool.tile([128, 512], f32, tag="ob", name="ob")
                    if copy_eng is None:
                        nc.scalar.copy(out=ob[:], in_=ps)
                    else:
                        copy_eng.tensor_copy(out=ob[:], in_=ps)
                    nc.sync.dma_start(
                        out_d[gt * 128:(gt + 1) * 128,
                              nck * 512:(nck + 1) * 512],
                        ob[:],
                    )

        # Emission order: projections stream in chunk order; each attention
        # chunk is emitted as soon as its projections exist, EXCEPT chunk 0
        # (the smallest, 4 j-tiles) which is saved for the tail so the
        # ACT-bound final attention stretch is as short as possible.
        pieces = issue_x(0)
        for c in range(N_CH - 1):
            qc = proj_chunk(c, pieces)
            pieces = issue_x(c + 1)
            yc = ypool.tile([128, H_LOC, TCH], f32r, tag="yc", name="yc")
            attn_span(c * TCH, TCH, qc, 0, yc)
            cproj_span(c * TCH, TCH, yc, 0)
        # last chunk: q projection + rope first, then its non-diagonal
        # attention (kt/vt chunks 0..2) overlaps the k/v projections
        c = N_CH - 1
        qc = proj_chunk(c, pieces, only="q")
        yc = ypool.tile([128, H_LOC, TCH], f32r, tag="yc", name="yc")
        st = attn_span(c * TCH, TCH, qc, 0, yc, jt_hi=4 * c)
        proj_chunk(c, pieces, only="kv", qc=qc)
        attn_span(c * TCH, TCH, qc, 0, yc, jt_lo=4 * c, state=st)
        cproj_span(c * TCH, TCH, yc, 0)

    nc.compile()
    _CACHE["nc"] = nc
    return nc


def host_inputs(x, Wq, Wk, Wv, Wo):
    """Per-core input dicts (host-side shard + transpose + table prep)."""
    x2 = np.ascontiguousarray(x.reshape(T, D).T).astype(np.float32)  # (D, T)

    half = DH // 2  # 64
    af = (1.0 / 1024.0) ** np.linspace(0.0, 1.0, DH // 4, dtype=np.float32)
    af = np.concatenate([af, np.zeros(DH // 4, np.float32)])         # (64,)
    theta = np.arange(T, dtype=np.float32)[:, None] * af[None, :]    # (T, 64)
    cos = np.cos(theta).T.astype(np.float32)                         # (64, T)
    sin = np.sin(theta).T.astype(np.float32)
    ctab = np.concatenate([cos, cos], axis=0)                        # (128, T)
    stab = np.concatenate([sin, -sin], axis=0)

    roll = np.zeros((128, 128), np.float32)
    for p in range(128):
        roll[p, (p + half) % 128] = 1.0
    ones = np.ones((128, 128), np.float32)
    tri = np.triu(np.ones((128, 128), np.float32))  # tri[j, i] = i >= j

    shared = {
        "xT": x2, "ctab": ctab, "stab": stab,
        "roll": roll, "ones": ones, "tri": tri,
    }
    in_maps = []
    for c in range(N_CORES):
        sl = slice(c * HD_LOC, (c + 1) * HD_LOC)
        in_maps.append({
            **shared,
            "wqT": np.ascontiguousarray(Wq[sl, :].T),
            "wkT": np.ascontiguousarray(Wk[sl, :].T),
            "wvT": np.ascontiguousarray(Wv[sl, :].T),
            "woT": np.ascontiguousarray((Wo[:, sl] / 3.0).T),
        })
    return in_maps


def _get_runner():
    """Build the program + a persistent jitted SPMD executable (once)."""
    if "runner" in _CACHE:
        return _CACHE["runner"]

    import jax
    import concourse.mybir as mybir
    from concourse.bass2jax import (
        _bass_exec_p,
        install_neuronx_cc_hook,
        partition_id_tensor,
    )
    from jax.experimental.shard_map import shard_map
    from jax.sharding import Mesh, PartitionSpec

    nc = build_program()
    install_neuronx_cc_hook()
    assert nc.dbg_addr is None
    pid_name = nc.partition_id_tensor.name if nc.partition_id_tensor else None

    in_names, out_names, out_avals, zero_outs = [], [], [], []
    for alloc in nc.m.functions[0].allocations:
        if not isinstance(alloc, mybir.MemoryLocationSet):
            continue
        name = alloc.memorylocations[0].name
        if alloc.kind == "ExternalInput":
            if name != pid_name:
                in_names.append(name)
        elif alloc.kind == "ExternalOutput":
            out_names.append(name)
            shape = tuple(alloc.tensor_shape)
            dtype = mybir.dt.np(alloc.dtype)
            out_avals.append(jax.core.ShapedArray(shape, dtype))
            zero_outs.append(np.zeros(shape, dtype))
    n_params = len(in_names)
    all_names = list(in_names) + list(out_names)
    if pid_name is not None:
        all_names.append(pid_name)
    donate = tuple(range(n_params, n_params + len(out_names)))

    def _body(*args):
        operands = list(args)
        if pid_name is not None:
            operands.append(partition_id_tensor())
        outs = _bass_exec_p.bind(
            *operands,
            out_avals=tuple(out_avals),
            in_names=tuple(all_names),
            out_names=tuple(out_names),
            lowering_input_output_aliases=(),
            sim_require_finite=True,
            sim_require_nnan=True,
            nc=nc,
        )
        return tuple(outs)

    devices = jax.devices()[:N_CORES]
    mesh = Mesh(np.asarray(devices), ("core",))
    in_specs = (PartitionSpec("core"),) * (n_params + len(out_names))
    out_specs = (PartitionSpec("core"),) * len(out_names)
    fn = jax.jit(
        shard_map(_body, mesh=mesh, in_specs=in_specs, out_specs=out_specs,
                  check_rep=False),
        donate_argnums=donate,
        keep_unused=True,
    )
    runner = (fn, in_names, out_names, out_avals, zero_outs)
    _CACHE["runner"] = runner
    return runner


def run_spmd(in_maps):
    """Execute the SPMD program; returns per-core output dicts."""
    fn, in_names, out_names, out_avals, zero_outs = _get_runner()
    concat_in = [
        np.concatenate([np.asarray(in_maps[c][n]) for c in range(N_CORES)], axis=0)
        for n in in_names
    ]
    concat_zeros = [
        np.zeros((N_CORES * z.shape[0], *z.shape[1:]), z.dtype) for z in zero_outs
    ]
    out_arrs = fn(*concat_in, *concat_zeros)
    return [
        {n: np.asarray(out_arrs[i]).reshape(N_CORES, *out_avals[i].shape)[c]
         for i, n in enumerate(out_names)}
        for c in range(N_CORES)
    ]


def kernel(x, Wq, Wk, Wv, Wo):
    in_maps = host_inputs(np.asarray(x), np.asarray(Wq), np.asarray(Wk),
                          np.asarray(Wv), np.asarray(Wo))
    results = run_spmd(in_maps)
    out = results[0]["outp"].astype(np.float64)
    for c in range(1, N_CORES):
        out += results[c]["outp"]
    return out.astype(np.float32).reshape(1, T, D)
